# revision 13
# baseline (speedup 1.0000x reference)
"""Trainium2 Bass kernel for nn_Classifier (segment mean-pool + tiny MLP head).

Pipeline (matches the jax reference):
  pooled[g] = mean of features over nodes with batch id g   (2048 graphs)
  out = LeakyReLU(LayerNorm(pooled @ W1 + b1)) @ W2 + b2    -> [2048, 1]

Sharding: batch ids are sorted, so nodes split across the 8 cores at
segment-block boundaries — core i owns graphs [256i, 256i+256) (two
128-segment regions) and exactly the nodes belonging to them. Segment sums
are disjoint per core, so no collective is needed; the host concatenates the
8 per-core [256]-sized outputs.

Per-core compute is a two-stage PE-only reduction with no runtime one-hot
generation (the vector/gpsimd is_equal one-hots were the measured bottleneck
of the previous version):
  stage 1: each 1024-node super-tile is summed into 128 group-sums (groups of
           8 consecutive nodes) via 8 matmuls whose stationaries are FIXED
           0/1 matrices uploaded once (S_k maps subtile k's 128 nodes to
           groups 16k..16k+16).
  stage 2: one matmul per super-tile scatters the 128 group-sums into the
           region's 128 segment rows using a host-built one-hot (the host
           knows every segment boundary), accumulating in PSUM. The one-hot
           carries 1/count instead of 1, so PSUM accumulates segment MEANS
           directly and no per-partition divide is needed.
The host pads each segment to start on a group (8-node) boundary (~1.5%
zero-pad), so every group belongs to exactly one segment. Features travel as
fp16 (halves HBM traffic vs fp32; verified end-to-end rel err ~4e-4 vs the
2e-2 gate). Segment counts come from a host bincount.

Scheduling notes: the 2MB one-hot upload goes on the vector engine's DMA
queue so the sync queue's completion counter (which gates the first PE work)
only covers the tiny pvec/smat uploads and the first feature chunks. Stage-2
matmuls are emitted 3 super-tiles behind stage 1 so the in-order PE queue
never stalls on the activation-engine PSUM->SBUF copies. Each region's MLP
head is emitted as soon as that region's stage-2 accumulation stops, hiding
region 0's head under region 1's feature stream.
"""

from contextlib import ExitStack

import numpy as np

import concourse.bass as bass
import concourse.mybir as mybir
import concourse.tile as tile
from concourse.bass_utils import run_bass_kernel_spmd

# ---------------------------------------------------------------------------
# Workaround: this walrus build rejects instructions carrying more than one
# semaphore wait ("Too many sync wait commands"), but Tile's semaphore
# assignment freely attaches several. After the TileContext has lowered the
# program, split any excess waits onto same-engine nops inserted right before
# the instruction (semantics are identical: all waits are monotonic and must
# hold before the instruction issues).
_MAX_WAITS = 1


def _split_excess_waits(nc: "bass.Bass", max_waits: int = _MAX_WAITS) -> None:
    ctr = 0
    for f in nc.m.functions:
        for b in f.blocks:
            out = []
            for inst in b.instructions:
                si = inst.sync_info
                waits = list(si.on_wait) if (si is not None and si.on_wait) else []
                if len(waits) > max_waits:
                    keep = waits[-max_waits:]
                    extra = waits[:-max_waits]
                    # On the PE queue the carrier must be a DRAIN: silicon
                    # promotes waitless LDWEIGHTS past in-flight work, so a
                    # plain nop's wait can be bypassed (walrus attaches a
                    # matmul's waits to its LDWEIGHTS — stripping them onto a
                    # nop re-opens that race). A drain fully serializes.
                    is_pe = inst.engine == mybir.EngineType.PE
                    for i in range(0, len(extra), max_waits):
                        ctr += 1
                        if is_pe:
                            nop = mybir.InstDrain(
                                name=f"waitsplit_drain_{ctr}", ins=[], outs=[],
                                engine=inst.engine,
                            )
                        else:
                            nop = mybir.InstNoOp(
                                name=f"waitsplit_nop_{ctr}", ins=[], outs=[],
                                engine=inst.engine,
                            )
                        nop.sync_info = mybir.SyncInfo(
                            on_wait=extra[i : i + max_waits], on_update=[]
                        )
                        nc.register_instruction(nop)
                        out.append(nop)
                    inst.sync_info = mybir.SyncInfo(
                        on_wait=keep, on_update=list(si.on_update or [])
                    )
                out.append(inst)
            b.instructions = out
# ---------------------------------------------------------------------------

N_CORES = 8
NUM_GRAPHS = 2048
SEGS_PER_CORE = NUM_GRAPHS // N_CORES  # 256
N_BLOCKS = 16  # 128-segment blocks; 2 per core (= regions)
D = 256
G = 8  # nodes per group (segment starts padded to multiples of G)
ST_NODES = 1024  # nodes per super-tile (8 subtiles x 128)
K_SUB = 8
LN_EPS = 1e-5
NEG_SLOPE = 0.01

_F16 = mybir.dt.float16
_F32 = mybir.dt.float32
_ALU = mybir.AluOpType

# Test/debug hooks: set PROFILE=True before calling kernel() to request an
# NTFF trace; the BassKernelResults lands in LAST_RESULT.
PROFILE = False
PROFILE_DIR = None
LAST_RESULT = None


def _build_program(r_st: int) -> bass.Bass:
    """r_st: super-tiles per 128-segment region (2 regions per core)."""
    NT = 2 * r_st  # super-tiles per core

    nc = bass.Bass("TRN2", debug=False)
    # super-tiles travel in PAIRS: 8KB contiguous per partition per DMA, which
    # halves the per-descriptor queue overhead vs 4KB lines
    feat = nc.dram_tensor("feat", [r_st * 128, 2 * K_SUB * D], _F16, kind="ExternalInput").ap()
    s_d = nc.dram_tensor("smat", [128, K_SUB * 128], _F16, kind="ExternalInput").ap()
    oh_d = nc.dram_tensor("oh", [128, NT * 128], _F16, kind="ExternalInput").ap()
    ident_d = nc.dram_tensor("ident", [128, 128], _F32, kind="ExternalInput").ap()
    w1aug_d = nc.dram_tensor("w1aug", [D + 1, 128], _F32, kind="ExternalInput").ap()
    pvec_d = nc.dram_tensor("pvec", [1, 385], _F32, kind="ExternalInput").ap()
    out_d = nc.dram_tensor("out", [2, 128], _F32, kind="ExternalOutput").ap()

    SKEW = 3  # stage-2 trails stage-1 by this many super-tiles on the PE queue
    HEAD_SKEW = 2  # region head trails its last stage-2 matmul

    with tile.TileContext(nc) as tc, ExitStack() as ctx:
        cpool = ctx.enter_context(tc.tile_pool(name="consts", bufs=1))
        fpool = ctx.enter_context(tc.tile_pool(name="feat", bufs=8))
        gpool = ctx.enter_context(tc.tile_pool(name="gp", bufs=2, space="PSUM"))
        gspool = ctx.enter_context(tc.tile_pool(name="gs", bufs=6))
        acc = ctx.enter_context(tc.tile_pool(name="acc", bufs=1, space="PSUM"))
        ppool = ctx.enter_context(tc.tile_pool(name="pw", bufs=1, space="PSUM"))
        spool = ctx.enter_context(tc.tile_pool(name="small", bufs=2))

        # tiny uploads first on the sync queue: its completion counter gates
        # the first PE work, so nothing fat may precede these.
        pv = cpool.tile([1, 385], _F32, tag="pv")
        nc.sync.dma_start(out=pv[:], in_=pvec_d[:])
        s_t = cpool.tile([128, K_SUB * 128], _F16, tag="smat")
        nc.sync.dma_start(out=s_t[:], in_=s_d[:])
        # fat one-hot upload rides the scalar engine's DMA queue (it issues no
        # other DMAs, so its completion counter gates only stage-2 matmuls)
        oh_t = cpool.tile([128, NT * 128], _F16, tag="oh")
        nc.scalar.dma_start(out=oh_t[:], in_=oh_d[:])
        ident_t = cpool.tile([128, 128], _F32, tag="ident")
        w1a = cpool.tile([128, 128], _F32, tag="w1a")
        w1b = cpool.tile([128, 128], _F32, tag="w1b")
        w1c = cpool.tile([1, 128], _F32, tag="w1c")
        ones_row = cpool.tile([1, 256], _F32, tag="ones")
        nc.vector.memset(ones_row[:], 1.0)
        epsc = cpool.tile([128, 1], _F32, tag="epsc")
        nc.vector.memset(epsc[:], LN_EPS)
        bc = cpool.tile([128, 385], _F32, tag="bcs")

        sums = [acc.tile([128, D], _F32, tag=f"sum{r}", name=f"sum{r}") for r in range(2)]
        ptT = [spool.tile([128, 256], _F32, tag=f"ptT{fb}", name=f"ptT{fb}") for fb in range(2)]

        def emit_head_transposes(r):
            # sums[r] already holds pooled means (1/count folded into oh)
            pooled = spool.tile([128, 256], _F32, tag=f"pooled{r}", name=f"pooled{r}")
            nc.scalar.copy(pooled[:], sums[r][:])
            for fb in range(2):
                tp = ppool.tile([128, 128], _F32, tag="tp")
                nc.tensor.transpose(
                    out=tp[:], in_=pooled[:, fb * 128 : (fb + 1) * 128],
                    identity=ident_t[:],
                )
                nc.scalar.copy(ptT[fb][:, r * 128 : (r + 1) * 128], tp[:])

        def emit_head(m):
            # h = pooled @ W1 + b1; LayerNorm; LeakyReLU; @ W2 + b2
            msl = slice(m * 128, (m + 1) * 128)
            h_ps = ppool.tile([128, 128], _F32, tag="h")
            nc.tensor.matmul(
                out=h_ps[:], lhsT=ptT[0][:, msl], rhs=w1a[:], start=True, stop=False
            )
            nc.tensor.matmul(
                out=h_ps[:], lhsT=ptT[1][:, msl], rhs=w1b[:], start=False, stop=False
            )
            nc.tensor.matmul(
                out=h_ps[:], lhsT=ones_row[:, msl], rhs=w1c[:], start=False, stop=True
            )

            musum = spool.tile([128, 1], _F32, tag="musum")
            nc.vector.tensor_reduce(
                out=musum[:], in_=h_ps[:], axis=mybir.AxisListType.X, op=_ALU.add
            )
            mu = spool.tile([128, 1], _F32, tag="mu")
            nc.vector.tensor_scalar(
                out=mu[:], in0=musum[:], scalar1=1.0 / 128, scalar2=None, op0=_ALU.mult
            )
            hc = spool.tile([128, 128], _F32, tag="hc")
            nc.vector.tensor_scalar(
                out=hc[:], in0=h_ps[:], scalar1=mu[:], scalar2=None, op0=_ALU.subtract
            )
            sq = spool.tile([128, 128], _F32, tag="sq")
            ssq = spool.tile([128, 1], _F32, tag="ssq")
            nc.vector.scalar_tensor_tensor(
                out=sq[:], in0=hc[:], scalar=1.0, in1=hc[:],
                op0=_ALU.mult, op1=_ALU.mult, accum_out=ssq[:],
            )
            std = spool.tile([128, 1], _F32, tag="std")
            nc.scalar.activation(
                std[:], ssq[:], mybir.ActivationFunctionType.Sqrt,
                bias=epsc[:], scale=1.0 / 128,
            )
            rstd = spool.tile([128, 1], _F32, tag="rstd")
            nc.vector.reciprocal(rstd[:], std[:])
            y = spool.tile([128, 128], _F32, tag="y")
            nc.vector.scalar_tensor_tensor(
                out=y[:], in0=hc[:], scalar=rstd[:], in1=bc[:, 0:128],
                op0=_ALU.mult, op1=_ALU.mult,
            )
            y2 = spool.tile([128, 128], _F32, tag="y2")
            nc.vector.tensor_tensor(out=y2[:], in0=y[:], in1=bc[:, 128:256],
                                    op=_ALU.add)
            yl = spool.tile([128, 128], _F32, tag="yl")
            nc.vector.scalar_tensor_tensor(
                out=yl[:], in0=y2[:], scalar=NEG_SLOPE, in1=y2[:],
                op0=_ALU.mult, op1=_ALU.max,
            )
            prod = spool.tile([128, 128], _F32, tag="prod")
            oc = spool.tile([128, 1], _F32, tag="oc")
            nc.vector.scalar_tensor_tensor(
                out=prod[:], in0=yl[:], scalar=1.0, in1=bc[:, 256:384],
                op0=_ALU.mult, op1=_ALU.mult, accum_out=oc[:],
            )
            ofin = spool.tile([128, 1], _F32, tag="ofin")
            nc.vector.tensor_scalar(
                out=ofin[:], in0=oc[:], scalar1=bc[:, 384:385], scalar2=None,
                op0=_ALU.add,
            )
            nc.sync.dma_start(out=out_d[m, :], in_=ofin[:])

        # ---- main stream: two-stage segment means ----
        gs_tiles = [None] * NT
        ft = None
        for idx in range(NT + SKEW):
            if idx < NT:
                st = idx
                if st % 2 == 0:
                    t = st // 2
                    ft = fpool.tile([128, 2 * K_SUB * D], _F16, tag="ft")
                    dma_eng = nc.sync if t % 2 == 0 else nc.gpsimd
                    dma_eng.dma_start(out=ft[:], in_=feat[t * 128 : (t + 1) * 128, :])
                c = st % 2
                gp = gpool.tile([128, D], _F32, tag="gp")
                for k in range(K_SUB):
                    kk = c * K_SUB + k
                    nc.tensor.matmul(
                        out=gp[:],
                        lhsT=s_t[:, k * 128 : (k + 1) * 128],
                        rhs=ft[:, kk * D : (kk + 1) * D],
                        start=(k == 0),
                        stop=(k == K_SUB - 1),
                    )
                gs = gspool.tile([128, D], _F16, tag="gs")
                nc.scalar.copy(gs[:], gp[:])
                gs_tiles[st] = gs
            if idx == 6:
                # head-only constants: uploaded behind the first feature
                # chunks so they never gate the stream
                nc.sync.dma_start(out=ident_t[:], in_=ident_d[:])
                nc.sync.dma_start(out=w1a[:], in_=w1aug_d[0:128, :])
                nc.sync.dma_start(out=w1b[:], in_=w1aug_d[128:256, :])
                nc.sync.dma_start(out=w1c[:], in_=w1aug_d[256:257, :])
                # broadcast [gamma | beta | W2 | b2] to all 128 partitions —
                # emitted here so it doesn't sit at the head of the PE queue
                bc_ps = ppool.tile([128, 385], _F32, tag="bc")
                nc.tensor.matmul(
                    out=bc_ps[:], lhsT=ones_row[:, 0:128], rhs=pv[:],
                    start=True, stop=True,
                )
                nc.scalar.copy(bc[:], bc_ps[:])
            if idx >= SKEW:
                # stage 2, trailing so the in-order PE queue never stalls on
                # the activation-engine PSUM->SBUF copy or the oh upload
                st2 = idx - SKEW
                r2, stl = divmod(st2, r_st)
                nc.tensor.matmul(
                    out=sums[r2][:],
                    lhsT=oh_t[:, st2 * 128 : (st2 + 1) * 128],
                    rhs=gs_tiles[st2][:],
                    start=(stl == 0),
                    stop=(stl == r_st - 1),
                )
                gs_tiles[st2] = None
            # region-0 head hides under region 1's stream
            if idx == r_st - 1 + SKEW + HEAD_SKEW:
                emit_head_transposes(0)
            if idx == r_st - 1 + SKEW + 2 * HEAD_SKEW:
                emit_head(0)

        emit_head_transposes(1)
        emit_head(1)

    _split_excess_waits(nc)
    return nc


def _prep_inputs(features, batch):
    """Group-aligned padded layout + per-core arrays.

    Returns (feat_cores [8, NT*128, 2048] f16, oh_cores [8, 128, NT*128] f16,
    r_st).
    """
    feats16 = np.asarray(features).astype(np.float16)
    seg = np.asarray(batch).astype(np.int64)
    n = seg.shape[0]
    counts = np.bincount(seg, minlength=NUM_GRAPHS)
    bnd = np.zeros(NUM_GRAPHS + 1, np.int64)
    bnd[1:] = np.cumsum(counts)

    # each segment starts at a multiple of G inside its 128-segment block
    pad_counts = ((counts + G - 1) // G) * G
    block_of_seg = np.arange(NUM_GRAPHS) // 128
    # per-block padded totals and r_st (shared by all cores: one SPMD program)
    blk_tot = np.zeros(N_BLOCKS, np.int64)
    np.add.at(blk_tot, block_of_seg, pad_counts)
    r_st = int(np.max((blk_tot + ST_NODES - 1) // ST_NODES))
    r_st += r_st % 2  # pairs of super-tiles share one DMA
    cap = r_st * ST_NODES  # padded node slots per block

    # start slot of each segment inside its block
    cum = np.cumsum(pad_counts)
    seg_start = cum - pad_counts
    blk_base = np.zeros(NUM_GRAPHS, np.int64)
    first_seg = np.arange(0, NUM_GRAPHS, 128)
    blk_base[first_seg] = seg_start[first_seg]
    blk_base = np.maximum.accumulate(blk_base)  # block-start offset per seg
    seg_start_local = seg_start - blk_base

    # scatter nodes into the padded [16, cap] layout
    rank = np.arange(n) - bnd[seg]
    dest = block_of_seg[seg] * cap + seg_start_local[seg] + rank
    fpad = np.zeros((N_BLOCKS * cap, D), np.float16)
    fpad[dest] = feats16
    segpad = np.full(N_BLOCKS * cap, -1, np.int64)
    segpad[dest] = seg

    # permute to the on-chip paired-super-tile layout: slot
    # (st*1024 + k*128 + p) lands at row (st//2)*128+p, col block
    # (st%2)*8+k  ->  [blk, r_st//2, 128, 2, 8, 256] (8KB per partition row)
    feat_blocks = (
        fpad.reshape(N_BLOCKS, r_st // 2, 2, K_SUB, 128, D)
        .transpose(0, 1, 4, 2, 3, 5)
        .reshape(N_BLOCKS, (r_st // 2) * 128, 2 * K_SUB * D)
    )
    feat_cores = np.ascontiguousarray(
        feat_blocks.reshape(N_CORES, r_st * 128, 2 * K_SUB * D)
    )

    # group segment ids: group g of block b = slots [8g, 8g+8) (uniform by
    # construction; first slot of a non-empty group is always a real node)
    gseg = segpad[::G].reshape(N_BLOCKS, r_st * 128)
    gseg_local = gseg - 128 * np.arange(N_BLOCKS)[:, None]  # pad rows -> <0
    # scaled one-hot [blk, st*128+g, s]: 1/count so PSUM accumulates means;
    # transpose to SBUF layout [blk, g(128), st, s]
    rec = (1.0 / np.maximum(counts, 1)).reshape(N_BLOCKS, 128)
    oh = (
        (gseg_local[:, :, None] == np.arange(128)[None, None, :])
        * rec[:, None, :]
    ).astype(np.float16)
    oh = (
        oh.reshape(N_BLOCKS, r_st, 128, 128)
        .transpose(0, 2, 1, 3)
        .reshape(N_BLOCKS, 128, r_st * 128)
    )
    # core i holds blocks 2i (region 0) and 2i+1 (region 1) side by side
    oh_cores = np.ascontiguousarray(
        oh.reshape(N_CORES, 2, 128, r_st * 128)
        .transpose(0, 2, 1, 3)
        .reshape(N_CORES, 128, 2 * r_st * 128)
    )

    return feat_cores, oh_cores, r_st


def kernel(features, batch, W1, b1, gamma, beta, W2, b2):
    feat_cores, oh_cores, r_st = _prep_inputs(features, batch)

    # fixed stage-1 stationaries: S[p, k, q] = 1 iff q == 16k + p//8
    p = np.arange(128)
    smat = np.zeros((128, K_SUB, 128), np.float16)
    for k in range(K_SUB):
        smat[p, k, 16 * k + p // G] = 1.0
    smat = smat.reshape(128, K_SUB * 128)

    ident = np.eye(128, dtype=np.float32)
    w1aug = np.concatenate(
        [np.asarray(W1, np.float32), np.asarray(b1, np.float32)[None, :]], axis=0
    )
    pvec = np.concatenate(
        [
            np.asarray(gamma, np.float32).ravel(),
            np.asarray(beta, np.float32).ravel(),
            np.asarray(W2, np.float32).ravel(),
            np.asarray(b2, np.float32).ravel(),
        ]
    )[None, :]

    nc = _build_program(r_st)
    in_maps = [
        {
            "feat": feat_cores[i],
            "smat": smat,
            "oh": oh_cores[i],
            "ident": ident,
            "w1aug": w1aug,
            "pvec": pvec,
        }
        for i in range(N_CORES)
    ]
    res = run_bass_kernel_spmd(
        nc, in_maps, list(range(N_CORES)), trace=PROFILE, tmpdir=PROFILE_DIR
    )
    global LAST_RESULT
    LAST_RESULT = res
    out = np.concatenate(
        [res.results[i]["out"].reshape(SEGS_PER_CORE) for i in range(N_CORES)]
    )
    return out.reshape(NUM_GRAPHS, 1).astype(np.float32)


# revision 17
# speedup vs baseline: 1.0238x; 1.0238x over previous
"""Trainium2 Bass kernel for nn_Classifier (segment mean-pool + tiny MLP head).

Pipeline (matches the jax reference):
  pooled[g] = mean of features over nodes with batch id g   (2048 graphs)
  out = LeakyReLU(LayerNorm(pooled @ W1 + b1)) @ W2 + b2    -> [2048, 1]

Sharding: batch ids are sorted, so nodes split across the 8 cores at
segment-block boundaries — core i owns graphs [256i, 256i+256) (two
128-segment regions) and exactly the nodes belonging to them. Segment sums
are disjoint per core, so no collective is needed; the host concatenates the
8 per-core [256]-sized outputs.

Per-core compute is a two-stage PE-only reduction with no runtime one-hot
generation (the vector/gpsimd is_equal one-hots were the measured bottleneck
of the previous version):
  stage 1: each 1024-node super-tile is summed into 128 group-sums (groups of
           8 consecutive nodes) via 8 matmuls whose stationaries are FIXED
           0/1 matrices uploaded once (S_k maps subtile k's 128 nodes to
           groups 16k..16k+16).
  stage 2: one matmul per super-tile scatters the 128 group-sums into the
           region's 128 segment rows using a host-built one-hot (the host
           knows every segment boundary), accumulating in PSUM. The one-hot
           carries 1/count instead of 1, so PSUM accumulates segment MEANS
           directly and no per-partition divide is needed.
The host pads each segment to start on a group (8-node) boundary (~1.5%
zero-pad), so every group belongs to exactly one segment. Features travel as
fp16 (halves HBM traffic vs fp32; verified end-to-end rel err ~4e-4 vs the
2e-2 gate). Segment counts come from a host bincount.

Scheduling notes: the 2MB one-hot upload goes on the vector engine's DMA
queue so the sync queue's completion counter (which gates the first PE work)
only covers the tiny pvec/smat uploads and the first feature chunks. Stage-2
matmuls are emitted 3 super-tiles behind stage 1 so the in-order PE queue
never stalls on the activation-engine PSUM->SBUF copies. Each region's MLP
head is emitted as soon as that region's stage-2 accumulation stops, hiding
region 0's head under region 1's feature stream.
"""

from contextlib import ExitStack

import numpy as np

import concourse.bass as bass
import concourse.mybir as mybir
import concourse.tile as tile
from concourse.bass_utils import run_bass_kernel_spmd

# ---------------------------------------------------------------------------
# Workaround: this walrus build rejects instructions carrying more than one
# semaphore wait ("Too many sync wait commands"), but Tile's semaphore
# assignment freely attaches several. After the TileContext has lowered the
# program, split any excess waits onto same-engine nops inserted right before
# the instruction (semantics are identical: all waits are monotonic and must
# hold before the instruction issues).
_MAX_WAITS = 1


def _split_excess_waits(nc: "bass.Bass", max_waits: int = _MAX_WAITS) -> None:
    ctr = 0
    for f in nc.m.functions:
        for b in f.blocks:
            out = []
            for inst in b.instructions:
                si = inst.sync_info
                waits = list(si.on_wait) if (si is not None and si.on_wait) else []
                if len(waits) > max_waits:
                    keep = waits[-max_waits:]
                    extra = waits[:-max_waits]
                    # On the PE queue the carrier must be a DRAIN: silicon
                    # promotes waitless LDWEIGHTS past in-flight work, so a
                    # plain nop's wait can be bypassed (walrus attaches a
                    # matmul's waits to its LDWEIGHTS — stripping them onto a
                    # nop re-opens that race). A drain fully serializes.
                    is_pe = inst.engine == mybir.EngineType.PE
                    for i in range(0, len(extra), max_waits):
                        ctr += 1
                        if is_pe:
                            nop = mybir.InstDrain(
                                name=f"waitsplit_drain_{ctr}", ins=[], outs=[],
                                engine=inst.engine,
                            )
                        else:
                            nop = mybir.InstNoOp(
                                name=f"waitsplit_nop_{ctr}", ins=[], outs=[],
                                engine=inst.engine,
                            )
                        nop.sync_info = mybir.SyncInfo(
                            on_wait=extra[i : i + max_waits], on_update=[]
                        )
                        nc.register_instruction(nop)
                        out.append(nop)
                    inst.sync_info = mybir.SyncInfo(
                        on_wait=keep, on_update=list(si.on_update or [])
                    )
                out.append(inst)
            b.instructions = out
# ---------------------------------------------------------------------------

N_CORES = 8
NUM_GRAPHS = 2048
SEGS_PER_CORE = NUM_GRAPHS // N_CORES  # 256
N_BLOCKS = 16  # 128-segment blocks; 2 per core (= regions)
D = 256
G = 8  # nodes per group (segment starts padded to multiples of G)
ST_NODES = 1024  # nodes per super-tile (8 subtiles x 128)
K_SUB = 8
LN_EPS = 1e-5
NEG_SLOPE = 0.01

_F16 = mybir.dt.float16
_F32 = mybir.dt.float32
_ALU = mybir.AluOpType

# Test/debug hooks: set PROFILE=True before calling kernel() to request an
# NTFF trace; the BassKernelResults lands in LAST_RESULT.
PROFILE = False
PROFILE_DIR = None
LAST_RESULT = None


def _build_program(r_st: int) -> bass.Bass:
    """r_st: super-tiles per 128-segment region (2 regions per core)."""
    NT = 2 * r_st  # super-tiles per core

    nc = bass.Bass("TRN2", debug=False)
    feat = nc.dram_tensor("feat", [NT * 128, K_SUB * D], _F16, kind="ExternalInput").ap()
    s_d = nc.dram_tensor("smat", [128, K_SUB * 128], _F16, kind="ExternalInput").ap()
    oh_d = nc.dram_tensor("oh", [128, NT * 128], _F16, kind="ExternalInput").ap()
    ident_d = nc.dram_tensor("ident", [128, 128], _F32, kind="ExternalInput").ap()
    w1aug_d = nc.dram_tensor("w1aug", [D + 1, 128], _F32, kind="ExternalInput").ap()
    pvec_d = nc.dram_tensor("pvec", [1, 385], _F32, kind="ExternalInput").ap()
    out_d = nc.dram_tensor("out", [2, 128], _F32, kind="ExternalOutput").ap()

    SKEW = 3  # stage-2 trails stage-1 by this many super-tiles on the PE queue
    HEAD_SKEW = 2  # region head trails its last stage-2 matmul

    with tile.TileContext(nc) as tc, ExitStack() as ctx:
        cpool = ctx.enter_context(tc.tile_pool(name="consts", bufs=1))
        fpool = ctx.enter_context(tc.tile_pool(name="feat", bufs=8))
        gpool = ctx.enter_context(tc.tile_pool(name="gp", bufs=2, space="PSUM"))
        gspool = ctx.enter_context(tc.tile_pool(name="gs", bufs=6))
        acc = ctx.enter_context(tc.tile_pool(name="acc", bufs=1, space="PSUM"))
        ppool = ctx.enter_context(tc.tile_pool(name="pw", bufs=1, space="PSUM"))
        spool = ctx.enter_context(tc.tile_pool(name="small", bufs=2))

        # tiny uploads first on the sync queue: its completion counter gates
        # the first PE work, so nothing fat may precede these.
        pv = cpool.tile([1, 385], _F32, tag="pv")
        nc.sync.dma_start(out=pv[:], in_=pvec_d[:])
        s_t = cpool.tile([128, K_SUB * 128], _F16, tag="smat")
        nc.sync.dma_start(out=s_t[:], in_=s_d[:])
        # fat one-hot upload rides the scalar engine's DMA queue (it issues no
        # other DMAs, so its completion counter gates only stage-2 matmuls)
        oh_t = cpool.tile([128, NT * 128], _F16, tag="oh")
        nc.scalar.dma_start(out=oh_t[:], in_=oh_d[:])
        ident_t = cpool.tile([128, 128], _F32, tag="ident")
        w1a = cpool.tile([128, 128], _F32, tag="w1a")
        w1b = cpool.tile([128, 128], _F32, tag="w1b")
        w1c = cpool.tile([1, 128], _F32, tag="w1c")
        ones_row = cpool.tile([1, 256], _F32, tag="ones")
        nc.vector.memset(ones_row[:], 1.0)
        epsc = cpool.tile([128, 1], _F32, tag="epsc")
        nc.vector.memset(epsc[:], LN_EPS)
        bc = cpool.tile([128, 385], _F32, tag="bcs")

        sums = [acc.tile([128, D], _F32, tag=f"sum{r}", name=f"sum{r}") for r in range(2)]
        ptT = [spool.tile([128, 256], _F32, tag=f"ptT{fb}", name=f"ptT{fb}") for fb in range(2)]

        def emit_head_transposes(r):
            # sums[r] already holds pooled means (1/count folded into oh)
            pooled = spool.tile([128, 256], _F32, tag=f"pooled{r}", name=f"pooled{r}")
            nc.scalar.copy(pooled[:], sums[r][:])
            for fb in range(2):
                tp = ppool.tile([128, 128], _F32, tag="tp")
                nc.tensor.transpose(
                    out=tp[:], in_=pooled[:, fb * 128 : (fb + 1) * 128],
                    identity=ident_t[:],
                )
                nc.scalar.copy(ptT[fb][:, r * 128 : (r + 1) * 128], tp[:])

        def emit_head(m):
            # h = pooled @ W1 + b1; LayerNorm; LeakyReLU; @ W2 + b2
            msl = slice(m * 128, (m + 1) * 128)
            h_ps = ppool.tile([128, 128], _F32, tag="h")
            nc.tensor.matmul(
                out=h_ps[:], lhsT=ptT[0][:, msl], rhs=w1a[:], start=True, stop=False
            )
            nc.tensor.matmul(
                out=h_ps[:], lhsT=ptT[1][:, msl], rhs=w1b[:], start=False, stop=False
            )
            nc.tensor.matmul(
                out=h_ps[:], lhsT=ones_row[:, msl], rhs=w1c[:], start=False, stop=True
            )

            musum = spool.tile([128, 1], _F32, tag="musum")
            nc.vector.tensor_reduce(
                out=musum[:], in_=h_ps[:], axis=mybir.AxisListType.X, op=_ALU.add
            )
            mu = spool.tile([128, 1], _F32, tag="mu")
            nc.vector.tensor_scalar(
                out=mu[:], in0=musum[:], scalar1=1.0 / 128, scalar2=None, op0=_ALU.mult
            )
            hc = spool.tile([128, 128], _F32, tag="hc")
            nc.vector.tensor_scalar(
                out=hc[:], in0=h_ps[:], scalar1=mu[:], scalar2=None, op0=_ALU.subtract
            )
            sq = spool.tile([128, 128], _F32, tag="sq")
            ssq = spool.tile([128, 1], _F32, tag="ssq")
            nc.vector.scalar_tensor_tensor(
                out=sq[:], in0=hc[:], scalar=1.0, in1=hc[:],
                op0=_ALU.mult, op1=_ALU.mult, accum_out=ssq[:],
            )
            std = spool.tile([128, 1], _F32, tag="std")
            nc.scalar.activation(
                std[:], ssq[:], mybir.ActivationFunctionType.Sqrt,
                bias=epsc[:], scale=1.0 / 128,
            )
            rstd = spool.tile([128, 1], _F32, tag="rstd")
            nc.vector.reciprocal(rstd[:], std[:])
            y = spool.tile([128, 128], _F32, tag="y")
            nc.vector.scalar_tensor_tensor(
                out=y[:], in0=hc[:], scalar=rstd[:], in1=bc[:, 0:128],
                op0=_ALU.mult, op1=_ALU.mult,
            )
            y2 = spool.tile([128, 128], _F32, tag="y2")
            nc.vector.tensor_tensor(out=y2[:], in0=y[:], in1=bc[:, 128:256],
                                    op=_ALU.add)
            yl = spool.tile([128, 128], _F32, tag="yl")
            nc.vector.scalar_tensor_tensor(
                out=yl[:], in0=y2[:], scalar=NEG_SLOPE, in1=y2[:],
                op0=_ALU.mult, op1=_ALU.max,
            )
            prod = spool.tile([128, 128], _F32, tag="prod")
            oc = spool.tile([128, 1], _F32, tag="oc")
            nc.vector.scalar_tensor_tensor(
                out=prod[:], in0=yl[:], scalar=1.0, in1=bc[:, 256:384],
                op0=_ALU.mult, op1=_ALU.mult, accum_out=oc[:],
            )
            ofin = spool.tile([128, 1], _F32, tag="ofin")
            nc.vector.tensor_scalar(
                out=ofin[:], in0=oc[:], scalar1=bc[:, 384:385], scalar2=None,
                op0=_ALU.add,
            )
            nc.sync.dma_start(out=out_d[m, :], in_=ofin[:])

        # ---- main stream: two-stage segment means ----
        gs_tiles = [None] * NT
        ft = None
        for idx in range(NT + SKEW):
            if idx < NT:
                st = idx
                ft = fpool.tile([128, K_SUB * D], _F16, tag="ft")
                dma_eng = nc.sync if st % 2 == 0 else nc.gpsimd
                dma_eng.dma_start(out=ft[:], in_=feat[st * 128 : (st + 1) * 128, :])
                gp = gpool.tile([128, D], _F32, tag="gp")
                for k in range(K_SUB):
                    nc.tensor.matmul(
                        out=gp[:],
                        lhsT=s_t[:, k * 128 : (k + 1) * 128],
                        rhs=ft[:, k * D : (k + 1) * D],
                        start=(k == 0),
                        stop=(k == K_SUB - 1),
                    )
                gs = gspool.tile([128, D], _F16, tag="gs")
                nc.scalar.copy(gs[:], gp[:])
                gs_tiles[st] = gs
            if idx == 6:
                # head-only constants: uploaded behind the first feature
                # chunks so they never gate the stream
                nc.sync.dma_start(out=ident_t[:], in_=ident_d[:])
                nc.sync.dma_start(out=w1a[:], in_=w1aug_d[0:128, :])
                nc.sync.dma_start(out=w1b[:], in_=w1aug_d[128:256, :])
                nc.sync.dma_start(out=w1c[:], in_=w1aug_d[256:257, :])
                # broadcast [gamma | beta | W2 | b2] to all 128 partitions —
                # emitted here so it doesn't sit at the head of the PE queue
                bc_ps = ppool.tile([128, 385], _F32, tag="bc")
                nc.tensor.matmul(
                    out=bc_ps[:], lhsT=ones_row[:, 0:128], rhs=pv[:],
                    start=True, stop=True,
                )
                nc.scalar.copy(bc[:], bc_ps[:])
            if idx >= SKEW:
                # stage 2, trailing so the in-order PE queue never stalls on
                # the activation-engine PSUM->SBUF copy or the oh upload
                st2 = idx - SKEW
                r2, stl = divmod(st2, r_st)
                nc.tensor.matmul(
                    out=sums[r2][:],
                    lhsT=oh_t[:, st2 * 128 : (st2 + 1) * 128],
                    rhs=gs_tiles[st2][:],
                    start=(stl == 0),
                    stop=(stl == r_st - 1),
                )
                gs_tiles[st2] = None
            # region-0 head hides under region 1's stream
            if idx == r_st - 1 + SKEW + HEAD_SKEW:
                emit_head_transposes(0)
            if idx == r_st - 1 + SKEW + 2 * HEAD_SKEW:
                emit_head(0)

        emit_head_transposes(1)
        emit_head(1)

    _split_excess_waits(nc)
    return nc


def _prep_inputs(features, batch):
    """Group-aligned padded layout + per-core arrays.

    Returns (feat_cores [8, NT*128, 2048] f16, oh_cores [8, 128, NT*128] f16,
    r_st).
    """
    feats16 = np.asarray(features).astype(np.float16)
    seg = np.asarray(batch).astype(np.int64)
    n = seg.shape[0]
    counts = np.bincount(seg, minlength=NUM_GRAPHS)
    bnd = np.zeros(NUM_GRAPHS + 1, np.int64)
    bnd[1:] = np.cumsum(counts)

    # each segment starts at a multiple of G inside its 128-segment block
    pad_counts = ((counts + G - 1) // G) * G
    block_of_seg = np.arange(NUM_GRAPHS) // 128
    # per-block padded totals and r_st (shared by all cores: one SPMD program)
    blk_tot = np.zeros(N_BLOCKS, np.int64)
    np.add.at(blk_tot, block_of_seg, pad_counts)
    r_st = int(np.max((blk_tot + ST_NODES - 1) // ST_NODES))
    cap = r_st * ST_NODES  # padded node slots per block

    # start slot of each segment inside its block
    cum = np.cumsum(pad_counts)
    seg_start = cum - pad_counts
    blk_base = np.zeros(NUM_GRAPHS, np.int64)
    first_seg = np.arange(0, NUM_GRAPHS, 128)
    blk_base[first_seg] = seg_start[first_seg]
    blk_base = np.maximum.accumulate(blk_base)  # block-start offset per seg
    seg_start_local = seg_start - blk_base

    # scatter nodes into the padded [16, cap] layout
    rank = np.arange(n) - bnd[seg]
    dest = block_of_seg[seg] * cap + seg_start_local[seg] + rank
    fpad = np.zeros((N_BLOCKS * cap, D), np.float16)
    fpad[dest] = feats16
    segpad = np.full(N_BLOCKS * cap, -1, np.int64)
    segpad[dest] = seg

    # permute to the on-chip super-tile layout: slot (st*1024 + k*128 + p)
    # lands at row st*128+p, cols k*256..  ->  [blk, r_st, 128, 8, 256]
    feat_blocks = (
        fpad.reshape(N_BLOCKS, r_st, K_SUB, 128, D)
        .transpose(0, 1, 3, 2, 4)
        .reshape(N_BLOCKS, r_st * 128, K_SUB * D)
    )
    feat_cores = np.ascontiguousarray(
        feat_blocks.reshape(N_CORES, 2 * r_st * 128, K_SUB * D)
    )

    # group segment ids: group g of block b = slots [8g, 8g+8) (uniform by
    # construction; first slot of a non-empty group is always a real node)
    gseg = segpad[::G].reshape(N_BLOCKS, r_st * 128)
    gseg_local = gseg - 128 * np.arange(N_BLOCKS)[:, None]  # pad rows -> <0
    # scaled one-hot [blk, st*128+g, s]: 1/count so PSUM accumulates means;
    # transpose to SBUF layout [blk, g(128), st, s]
    rec = (1.0 / np.maximum(counts, 1)).reshape(N_BLOCKS, 128)
    oh = (
        (gseg_local[:, :, None] == np.arange(128)[None, None, :])
        * rec[:, None, :]
    ).astype(np.float16)
    oh = (
        oh.reshape(N_BLOCKS, r_st, 128, 128)
        .transpose(0, 2, 1, 3)
        .reshape(N_BLOCKS, 128, r_st * 128)
    )
    # core i holds blocks 2i (region 0) and 2i+1 (region 1) side by side
    oh_cores = np.ascontiguousarray(
        oh.reshape(N_CORES, 2, 128, r_st * 128)
        .transpose(0, 2, 1, 3)
        .reshape(N_CORES, 128, 2 * r_st * 128)
    )

    return feat_cores, oh_cores, r_st


def kernel(features, batch, W1, b1, gamma, beta, W2, b2):
    feat_cores, oh_cores, r_st = _prep_inputs(features, batch)

    # fixed stage-1 stationaries: S[p, k, q] = 1 iff q == 16k + p//8
    p = np.arange(128)
    smat = np.zeros((128, K_SUB, 128), np.float16)
    for k in range(K_SUB):
        smat[p, k, 16 * k + p // G] = 1.0
    smat = smat.reshape(128, K_SUB * 128)

    ident = np.eye(128, dtype=np.float32)
    w1aug = np.concatenate(
        [np.asarray(W1, np.float32), np.asarray(b1, np.float32)[None, :]], axis=0
    )
    pvec = np.concatenate(
        [
            np.asarray(gamma, np.float32).ravel(),
            np.asarray(beta, np.float32).ravel(),
            np.asarray(W2, np.float32).ravel(),
            np.asarray(b2, np.float32).ravel(),
        ]
    )[None, :]

    nc = _build_program(r_st)
    in_maps = [
        {
            "feat": feat_cores[i],
            "smat": smat,
            "oh": oh_cores[i],
            "ident": ident,
            "w1aug": w1aug,
            "pvec": pvec,
        }
        for i in range(N_CORES)
    ]
    res = run_bass_kernel_spmd(
        nc, in_maps, list(range(N_CORES)), trace=PROFILE, tmpdir=PROFILE_DIR
    )
    global LAST_RESULT
    LAST_RESULT = res
    out = np.concatenate(
        [res.results[i]["out"].reshape(SEGS_PER_CORE) for i in range(N_CORES)]
    )
    return out.reshape(NUM_GRAPHS, 1).astype(np.float32)


# revision 18
# speedup vs baseline: 1.0431x; 1.0188x over previous
"""Trainium2 Bass kernel for nn_Classifier (segment mean-pool + tiny MLP head).

Pipeline (matches the jax reference):
  pooled[g] = mean of features over nodes with batch id g   (2048 graphs)
  out = LeakyReLU(LayerNorm(pooled @ W1 + b1)) @ W2 + b2    -> [2048, 1]

Sharding: batch ids are sorted, so nodes split across the 8 cores at
segment-block boundaries — core i owns graphs [256i, 256i+256) (two
128-segment regions) and exactly the nodes belonging to them. Segment sums
are disjoint per core, so no collective is needed; the host concatenates the
8 per-core [256]-sized outputs.

Per-core compute is a two-stage PE-only reduction with no runtime one-hot
generation (the vector/gpsimd is_equal one-hots were the measured bottleneck
of the previous version):
  stage 1: each 1024-node super-tile is summed into 128 group-sums (groups of
           8 consecutive nodes) via 8 matmuls whose stationaries are FIXED
           0/1 matrices uploaded once (S_k maps subtile k's 128 nodes to
           groups 16k..16k+16).
  stage 2: one matmul per super-tile scatters the 128 group-sums into the
           region's 128 segment rows using a host-built one-hot (the host
           knows every segment boundary), accumulating in PSUM. The one-hot
           carries 1/count instead of 1, so PSUM accumulates segment MEANS
           directly and no per-partition divide is needed.
The host pads each segment to start on a group (8-node) boundary (~1.5%
zero-pad), so every group belongs to exactly one segment. Features travel as
fp16 (halves HBM traffic vs fp32; verified end-to-end rel err ~4e-4 vs the
2e-2 gate). Segment counts come from a host bincount.

Scheduling notes: the 2MB one-hot upload goes on the vector engine's DMA
queue so the sync queue's completion counter (which gates the first PE work)
only covers the tiny pvec/smat uploads and the first feature chunks. Stage-2
matmuls are emitted 3 super-tiles behind stage 1 so the in-order PE queue
never stalls on the activation-engine PSUM->SBUF copies. Each region's MLP
head is emitted as soon as that region's stage-2 accumulation stops, hiding
region 0's head under region 1's feature stream.
"""

from contextlib import ExitStack

import ml_dtypes
import numpy as np

import concourse.bass as bass
import concourse.mybir as mybir
import concourse.tile as tile
from concourse.bass_utils import run_bass_kernel_spmd

# ---------------------------------------------------------------------------
# Workaround: this walrus build rejects instructions carrying more than one
# semaphore wait ("Too many sync wait commands"), but Tile's semaphore
# assignment freely attaches several. After the TileContext has lowered the
# program, split any excess waits onto same-engine nops inserted right before
# the instruction (semantics are identical: all waits are monotonic and must
# hold before the instruction issues).
_MAX_WAITS = 1


def _split_excess_waits(nc: "bass.Bass", max_waits: int = _MAX_WAITS) -> None:
    ctr = 0
    for f in nc.m.functions:
        for b in f.blocks:
            out = []
            for inst in b.instructions:
                si = inst.sync_info
                waits = list(si.on_wait) if (si is not None and si.on_wait) else []
                if len(waits) > max_waits:
                    keep = waits[-max_waits:]
                    extra = waits[:-max_waits]
                    # On the PE queue the carrier must be a DRAIN: silicon
                    # promotes waitless LDWEIGHTS past in-flight work, so a
                    # plain nop's wait can be bypassed (walrus attaches a
                    # matmul's waits to its LDWEIGHTS — stripping them onto a
                    # nop re-opens that race). A drain fully serializes.
                    is_pe = inst.engine == mybir.EngineType.PE
                    for i in range(0, len(extra), max_waits):
                        ctr += 1
                        if is_pe:
                            nop = mybir.InstDrain(
                                name=f"waitsplit_drain_{ctr}", ins=[], outs=[],
                                engine=inst.engine,
                            )
                        else:
                            nop = mybir.InstNoOp(
                                name=f"waitsplit_nop_{ctr}", ins=[], outs=[],
                                engine=inst.engine,
                            )
                        nop.sync_info = mybir.SyncInfo(
                            on_wait=extra[i : i + max_waits], on_update=[]
                        )
                        nc.register_instruction(nop)
                        out.append(nop)
                    inst.sync_info = mybir.SyncInfo(
                        on_wait=keep, on_update=list(si.on_update or [])
                    )
                out.append(inst)
            b.instructions = out
# ---------------------------------------------------------------------------

N_CORES = 8
NUM_GRAPHS = 2048
SEGS_PER_CORE = NUM_GRAPHS // N_CORES  # 256
N_BLOCKS = 16  # 128-segment blocks; 2 per core (= regions)
D = 256
G = 8  # nodes per group (segment starts padded to multiples of G)
ST_NODES = 1024  # nodes per super-tile (8 subtiles x 128)
K_SUB = 8
LN_EPS = 1e-5
NEG_SLOPE = 0.01

_F16 = mybir.dt.float16
_F8 = mybir.dt.float8e4
_F32 = mybir.dt.float32
_ALU = mybir.AluOpType

# Test/debug hooks: set PROFILE=True before calling kernel() to request an
# NTFF trace; the BassKernelResults lands in LAST_RESULT.
PROFILE = False
PROFILE_DIR = None
LAST_RESULT = None


def _build_program(r_st: int) -> bass.Bass:
    """r_st: super-tiles per 128-segment region (2 regions per core)."""
    NT = 2 * r_st  # super-tiles per core

    nc = bass.Bass("TRN2", debug=False)
    feat = nc.dram_tensor("feat", [NT * 128, K_SUB * D], _F16, kind="ExternalInput").ap()
    s_d = nc.dram_tensor("smat", [128, K_SUB * 128], _F16, kind="ExternalInput").ap()
    oh_d = nc.dram_tensor("oh", [128, NT * 128], _F8, kind="ExternalInput").ap()
    ident_d = nc.dram_tensor("ident", [128, 128], _F32, kind="ExternalInput").ap()
    w1aug_d = nc.dram_tensor("w1aug", [D + 1, 128], _F32, kind="ExternalInput").ap()
    pvec_d = nc.dram_tensor("pvec", [1, 385], _F32, kind="ExternalInput").ap()
    rec_d = nc.dram_tensor("rec", [128, 2], _F32, kind="ExternalInput").ap()
    out_d = nc.dram_tensor("out", [2, 128], _F32, kind="ExternalOutput").ap()

    SKEW = 3  # stage-2 trails stage-1 by this many super-tiles on the PE queue
    HEAD_SKEW = 2  # region head trails its last stage-2 matmul

    with tile.TileContext(nc) as tc, ExitStack() as ctx:
        cpool = ctx.enter_context(tc.tile_pool(name="consts", bufs=1))
        fpool = ctx.enter_context(tc.tile_pool(name="feat", bufs=8))
        gpool = ctx.enter_context(tc.tile_pool(name="gp", bufs=2, space="PSUM"))
        gspool = ctx.enter_context(tc.tile_pool(name="gs", bufs=6))
        acc = ctx.enter_context(tc.tile_pool(name="acc", bufs=1, space="PSUM"))
        ppool = ctx.enter_context(tc.tile_pool(name="pw", bufs=1, space="PSUM"))
        spool = ctx.enter_context(tc.tile_pool(name="small", bufs=2))

        # tiny uploads first on the sync queue: its completion counter gates
        # the first PE work, so nothing fat may precede these.
        pv = cpool.tile([1, 385], _F32, tag="pv")
        nc.sync.dma_start(out=pv[:], in_=pvec_d[:])
        rec_t = cpool.tile([128, 2], _F32, tag="rec")
        nc.sync.dma_start(out=rec_t[:], in_=rec_d[:])
        # smat + the fat one-hot ride the scalar engine's DMA queue (it issues
        # no other DMAs, so stage-1 releases on just the smat completion)
        s_t = cpool.tile([128, K_SUB * 128], _F16, tag="smat")
        nc.scalar.dma_start(out=s_t[:], in_=s_d[:])
        oh_t = cpool.tile([128, NT * 128], _F8, tag="oh")
        nc.scalar.dma_start(out=oh_t[:], in_=oh_d[:])
        ident_t = cpool.tile([128, 128], _F32, tag="ident")
        w1a = cpool.tile([128, 128], _F32, tag="w1a")
        w1b = cpool.tile([128, 128], _F32, tag="w1b")
        w1c = cpool.tile([1, 128], _F32, tag="w1c")
        ones_row = cpool.tile([1, 256], _F32, tag="ones")
        nc.vector.memset(ones_row[:], 1.0)
        epsc = cpool.tile([128, 1], _F32, tag="epsc")
        nc.vector.memset(epsc[:], LN_EPS)
        bc = cpool.tile([128, 385], _F32, tag="bcs")

        sums = [acc.tile([128, D], _F32, tag=f"sum{r}", name=f"sum{r}") for r in range(2)]
        ptT = [spool.tile([128, 256], _F32, tag=f"ptT{fb}", name=f"ptT{fb}") for fb in range(2)]

        def emit_head_transposes(r):
            # sums[r] already holds pooled means (1/count folded into oh)
            pooled = spool.tile([128, 256], _F32, tag=f"pooled{r}", name=f"pooled{r}")
            nc.scalar.mul(pooled[:], sums[r][:], rec_t[:, r : r + 1])
            for fb in range(2):
                tp = ppool.tile([128, 128], _F32, tag="tp")
                nc.tensor.transpose(
                    out=tp[:], in_=pooled[:, fb * 128 : (fb + 1) * 128],
                    identity=ident_t[:],
                )
                nc.scalar.copy(ptT[fb][:, r * 128 : (r + 1) * 128], tp[:])

        def emit_head(m):
            # h = pooled @ W1 + b1; LayerNorm; LeakyReLU; @ W2 + b2
            msl = slice(m * 128, (m + 1) * 128)
            h_ps = ppool.tile([128, 128], _F32, tag="h")
            nc.tensor.matmul(
                out=h_ps[:], lhsT=ptT[0][:, msl], rhs=w1a[:], start=True, stop=False
            )
            nc.tensor.matmul(
                out=h_ps[:], lhsT=ptT[1][:, msl], rhs=w1b[:], start=False, stop=False
            )
            nc.tensor.matmul(
                out=h_ps[:], lhsT=ones_row[:, msl], rhs=w1c[:], start=False, stop=True
            )

            musum = spool.tile([128, 1], _F32, tag="musum")
            nc.vector.tensor_reduce(
                out=musum[:], in_=h_ps[:], axis=mybir.AxisListType.X, op=_ALU.add
            )
            mu = spool.tile([128, 1], _F32, tag="mu")
            nc.vector.tensor_scalar(
                out=mu[:], in0=musum[:], scalar1=1.0 / 128, scalar2=None, op0=_ALU.mult
            )
            hc = spool.tile([128, 128], _F32, tag="hc")
            nc.vector.tensor_scalar(
                out=hc[:], in0=h_ps[:], scalar1=mu[:], scalar2=None, op0=_ALU.subtract
            )
            sq = spool.tile([128, 128], _F32, tag="sq")
            ssq = spool.tile([128, 1], _F32, tag="ssq")
            nc.vector.scalar_tensor_tensor(
                out=sq[:], in0=hc[:], scalar=1.0, in1=hc[:],
                op0=_ALU.mult, op1=_ALU.mult, accum_out=ssq[:],
            )
            std = spool.tile([128, 1], _F32, tag="std")
            nc.scalar.activation(
                std[:], ssq[:], mybir.ActivationFunctionType.Sqrt,
                bias=epsc[:], scale=1.0 / 128,
            )
            rstd = spool.tile([128, 1], _F32, tag="rstd")
            nc.vector.reciprocal(rstd[:], std[:])
            y = spool.tile([128, 128], _F32, tag="y")
            nc.vector.scalar_tensor_tensor(
                out=y[:], in0=hc[:], scalar=rstd[:], in1=bc[:, 0:128],
                op0=_ALU.mult, op1=_ALU.mult,
            )
            y2 = spool.tile([128, 128], _F32, tag="y2")
            nc.vector.tensor_tensor(out=y2[:], in0=y[:], in1=bc[:, 128:256],
                                    op=_ALU.add)
            yl = spool.tile([128, 128], _F32, tag="yl")
            nc.vector.scalar_tensor_tensor(
                out=yl[:], in0=y2[:], scalar=NEG_SLOPE, in1=y2[:],
                op0=_ALU.mult, op1=_ALU.max,
            )
            prod = spool.tile([128, 128], _F32, tag="prod")
            oc = spool.tile([128, 1], _F32, tag="oc")
            nc.vector.scalar_tensor_tensor(
                out=prod[:], in0=yl[:], scalar=1.0, in1=bc[:, 256:384],
                op0=_ALU.mult, op1=_ALU.mult, accum_out=oc[:],
            )
            ofin = spool.tile([128, 1], _F32, tag="ofin")
            nc.vector.tensor_scalar(
                out=ofin[:], in0=oc[:], scalar1=bc[:, 384:385], scalar2=None,
                op0=_ALU.add,
            )
            nc.sync.dma_start(out=out_d[m, :], in_=ofin[:])

        # ---- main stream: two-stage segment means ----
        gs_tiles = [None] * NT
        ft = None
        for idx in range(NT + SKEW):
            if idx < NT:
                st = idx
                ft = fpool.tile([128, K_SUB * D], _F16, tag="ft")
                dma_eng = nc.sync if st % 2 == 0 else nc.gpsimd
                dma_eng.dma_start(out=ft[:], in_=feat[st * 128 : (st + 1) * 128, :])
                gp = gpool.tile([128, D], _F32, tag="gp")
                for k in range(K_SUB):
                    nc.tensor.matmul(
                        out=gp[:],
                        lhsT=s_t[:, k * 128 : (k + 1) * 128],
                        rhs=ft[:, k * D : (k + 1) * D],
                        start=(k == 0),
                        stop=(k == K_SUB - 1),
                    )
                gs = gspool.tile([128, D], _F16, tag="gs")
                nc.scalar.copy(gs[:], gp[:])
                gs_tiles[st] = gs
            if idx == 6:
                # head-only constants: uploaded behind the first feature
                # chunks so they never gate the stream
                nc.sync.dma_start(out=ident_t[:], in_=ident_d[:])
                nc.sync.dma_start(out=w1a[:], in_=w1aug_d[0:128, :])
                nc.sync.dma_start(out=w1b[:], in_=w1aug_d[128:256, :])
                nc.sync.dma_start(out=w1c[:], in_=w1aug_d[256:257, :])
                # broadcast [gamma | beta | W2 | b2] to all 128 partitions —
                # emitted here so it doesn't sit at the head of the PE queue
                bc_ps = ppool.tile([128, 385], _F32, tag="bc")
                nc.tensor.matmul(
                    out=bc_ps[:], lhsT=ones_row[:, 0:128], rhs=pv[:],
                    start=True, stop=True,
                )
                nc.scalar.copy(bc[:], bc_ps[:])
            if idx >= SKEW:
                # stage 2, trailing so the in-order PE queue never stalls on
                # the activation-engine PSUM->SBUF copy or the oh upload
                st2 = idx - SKEW
                r2, stl = divmod(st2, r_st)
                nc.tensor.matmul(
                    out=sums[r2][:],
                    lhsT=oh_t[:, st2 * 128 : (st2 + 1) * 128],
                    rhs=gs_tiles[st2][:],
                    start=(stl == 0),
                    stop=(stl == r_st - 1),
                )
                gs_tiles[st2] = None
            # region-0 head hides under region 1's stream
            if idx == r_st - 1 + SKEW + HEAD_SKEW:
                emit_head_transposes(0)
            if idx == r_st - 1 + SKEW + 2 * HEAD_SKEW:
                emit_head(0)

        emit_head_transposes(1)
        emit_head(1)

    _split_excess_waits(nc)
    return nc


def _prep_inputs(features, batch):
    """Group-aligned padded layout + per-core arrays.

    Returns (feat_cores [8, NT*128, 2048] f16, oh_cores [8, 128, NT*128] f16,
    r_st).
    """
    feats16 = np.asarray(features).astype(np.float16)
    seg = np.asarray(batch).astype(np.int64)
    n = seg.shape[0]
    counts = np.bincount(seg, minlength=NUM_GRAPHS)
    bnd = np.zeros(NUM_GRAPHS + 1, np.int64)
    bnd[1:] = np.cumsum(counts)

    # each segment starts at a multiple of G inside its 128-segment block
    pad_counts = ((counts + G - 1) // G) * G
    block_of_seg = np.arange(NUM_GRAPHS) // 128
    # per-block padded totals and r_st (shared by all cores: one SPMD program)
    blk_tot = np.zeros(N_BLOCKS, np.int64)
    np.add.at(blk_tot, block_of_seg, pad_counts)
    r_st = int(np.max((blk_tot + ST_NODES - 1) // ST_NODES))
    cap = r_st * ST_NODES  # padded node slots per block

    # start slot of each segment inside its block
    cum = np.cumsum(pad_counts)
    seg_start = cum - pad_counts
    blk_base = np.zeros(NUM_GRAPHS, np.int64)
    first_seg = np.arange(0, NUM_GRAPHS, 128)
    blk_base[first_seg] = seg_start[first_seg]
    blk_base = np.maximum.accumulate(blk_base)  # block-start offset per seg
    seg_start_local = seg_start - blk_base

    # scatter nodes into the padded [16, cap] layout
    rank = np.arange(n) - bnd[seg]
    dest = block_of_seg[seg] * cap + seg_start_local[seg] + rank
    fpad = np.zeros((N_BLOCKS * cap, D), np.float16)
    fpad[dest] = feats16
    segpad = np.full(N_BLOCKS * cap, -1, np.int64)
    segpad[dest] = seg

    # permute to the on-chip super-tile layout: slot (st*1024 + k*128 + p)
    # lands at row st*128+p, cols k*256..  ->  [blk, r_st, 128, 8, 256]
    feat_blocks = (
        fpad.reshape(N_BLOCKS, r_st, K_SUB, 128, D)
        .transpose(0, 1, 3, 2, 4)
        .reshape(N_BLOCKS, r_st * 128, K_SUB * D)
    )
    feat_cores = np.ascontiguousarray(
        feat_blocks.reshape(N_CORES, 2 * r_st * 128, K_SUB * D)
    )

    # group segment ids: group g of block b = slots [8g, 8g+8) (uniform by
    # construction; first slot of a non-empty group is always a real node)
    gseg = segpad[::G].reshape(N_BLOCKS, r_st * 128)
    gseg_local = gseg - 128 * np.arange(N_BLOCKS)[:, None]  # pad rows -> <0
    # 0/1 one-hot [blk, st*128+g, s] (exact in fp8e4m3);
    # transpose to SBUF layout [blk, g(128), st, s]
    oh = (gseg_local[:, :, None] == np.arange(128)[None, None, :]).astype(
        ml_dtypes.float8_e4m3
    )
    oh = (
        oh.reshape(N_BLOCKS, r_st, 128, 128)
        .transpose(0, 2, 1, 3)
        .reshape(N_BLOCKS, 128, r_st * 128)
    )
    # core i holds blocks 2i (region 0) and 2i+1 (region 1) side by side
    oh_cores = np.ascontiguousarray(
        oh.reshape(N_CORES, 2, 128, r_st * 128)
        .transpose(0, 2, 1, 3)
        .reshape(N_CORES, 128, 2 * r_st * 128)
    )

    rec = (1.0 / np.maximum(counts, 1)).astype(np.float32)
    rec_cores = np.ascontiguousarray(
        rec.reshape(N_CORES, 2, 128).transpose(0, 2, 1)
    )
    return feat_cores, oh_cores, rec_cores, r_st


def kernel(features, batch, W1, b1, gamma, beta, W2, b2):
    feat_cores, oh_cores, rec_cores, r_st = _prep_inputs(features, batch)

    # fixed stage-1 stationaries: S[p, k, q] = 1 iff q == 16k + p//8
    p = np.arange(128)
    smat = np.zeros((128, K_SUB, 128), np.float16)
    for k in range(K_SUB):
        smat[p, k, 16 * k + p // G] = 1.0
    smat = smat.reshape(128, K_SUB * 128)

    ident = np.eye(128, dtype=np.float32)
    w1aug = np.concatenate(
        [np.asarray(W1, np.float32), np.asarray(b1, np.float32)[None, :]], axis=0
    )
    pvec = np.concatenate(
        [
            np.asarray(gamma, np.float32).ravel(),
            np.asarray(beta, np.float32).ravel(),
            np.asarray(W2, np.float32).ravel(),
            np.asarray(b2, np.float32).ravel(),
        ]
    )[None, :]

    nc = _build_program(r_st)
    in_maps = [
        {
            "feat": feat_cores[i],
            "smat": smat,
            "oh": oh_cores[i],
            "ident": ident,
            "w1aug": w1aug,
            "pvec": pvec,
            "rec": rec_cores[i],
        }
        for i in range(N_CORES)
    ]
    res = run_bass_kernel_spmd(
        nc, in_maps, list(range(N_CORES)), trace=PROFILE, tmpdir=PROFILE_DIR
    )
    global LAST_RESULT
    LAST_RESULT = res
    out = np.concatenate(
        [res.results[i]["out"].reshape(SEGS_PER_CORE) for i in range(N_CORES)]
    )
    return out.reshape(NUM_GRAPHS, 1).astype(np.float32)


# revision 21
# speedup vs baseline: 1.0546x; 1.0111x over previous
"""Trainium2 Bass kernel for nn_Classifier (segment mean-pool + tiny MLP head).

Pipeline (matches the jax reference):
  pooled[g] = mean of features over nodes with batch id g   (2048 graphs)
  out = LeakyReLU(LayerNorm(pooled @ W1 + b1)) @ W2 + b2    -> [2048, 1]

Sharding: batch ids are sorted, so nodes split across the 8 cores at
segment-block boundaries — core i owns graphs [256i, 256i+256) (two
128-segment regions) and exactly the nodes belonging to them. Segment sums
are disjoint per core, so no collective is needed; the host concatenates the
8 per-core [256]-sized outputs.

Per-core compute is a two-stage PE-only reduction with no runtime one-hot
generation (the vector/gpsimd is_equal one-hots were the measured bottleneck
of the previous version):
  stage 1: each 1024-node super-tile is summed into 128 group-sums (groups of
           8 consecutive nodes) via 8 matmuls whose stationaries are FIXED
           0/1 matrices uploaded once (S_k maps subtile k's 128 nodes to
           groups 16k..16k+16).
  stage 2: one matmul per super-tile scatters the 128 group-sums into the
           region's 128 segment rows using a host-built one-hot (the host
           knows every segment boundary), accumulating in PSUM. The one-hot
           carries 1/count instead of 1, so PSUM accumulates segment MEANS
           directly and no per-partition divide is needed.
The host pads each segment to start on a group (8-node) boundary (~1.5%
zero-pad), so every group belongs to exactly one segment. Features travel as
fp16 (halves HBM traffic vs fp32; verified end-to-end rel err ~4e-4 vs the
2e-2 gate). Segment counts come from a host bincount.

Scheduling notes: the 2MB one-hot upload goes on the vector engine's DMA
queue so the sync queue's completion counter (which gates the first PE work)
only covers the tiny pvec/smat uploads and the first feature chunks. Stage-2
matmuls are emitted 3 super-tiles behind stage 1 so the in-order PE queue
never stalls on the activation-engine PSUM->SBUF copies. Each region's MLP
head is emitted as soon as that region's stage-2 accumulation stops, hiding
region 0's head under region 1's feature stream.
"""

from contextlib import ExitStack

import ml_dtypes
import numpy as np

import concourse.bass as bass
import concourse.mybir as mybir
import concourse.tile as tile
from concourse.bass_utils import run_bass_kernel_spmd

# ---------------------------------------------------------------------------
# Workaround: this walrus build rejects instructions carrying more than one
# semaphore wait ("Too many sync wait commands"), but Tile's semaphore
# assignment freely attaches several. After the TileContext has lowered the
# program, split any excess waits onto same-engine nops inserted right before
# the instruction (semantics are identical: all waits are monotonic and must
# hold before the instruction issues).
_MAX_WAITS = 1


def _split_excess_waits(nc: "bass.Bass", max_waits: int = _MAX_WAITS) -> None:
    ctr = 0
    for f in nc.m.functions:
        for b in f.blocks:
            out = []
            for inst in b.instructions:
                si = inst.sync_info
                waits = list(si.on_wait) if (si is not None and si.on_wait) else []
                if len(waits) > max_waits:
                    keep = waits[-max_waits:]
                    extra = waits[:-max_waits]
                    # On the PE queue the carrier must be a DRAIN: silicon
                    # promotes waitless LDWEIGHTS past in-flight work, so a
                    # plain nop's wait can be bypassed (walrus attaches a
                    # matmul's waits to its LDWEIGHTS — stripping them onto a
                    # nop re-opens that race). A drain fully serializes.
                    is_pe = inst.engine == mybir.EngineType.PE
                    for i in range(0, len(extra), max_waits):
                        ctr += 1
                        if is_pe:
                            nop = mybir.InstDrain(
                                name=f"waitsplit_drain_{ctr}", ins=[], outs=[],
                                engine=inst.engine,
                            )
                        else:
                            nop = mybir.InstNoOp(
                                name=f"waitsplit_nop_{ctr}", ins=[], outs=[],
                                engine=inst.engine,
                            )
                        nop.sync_info = mybir.SyncInfo(
                            on_wait=extra[i : i + max_waits], on_update=[]
                        )
                        nc.register_instruction(nop)
                        out.append(nop)
                    inst.sync_info = mybir.SyncInfo(
                        on_wait=keep, on_update=list(si.on_update or [])
                    )
                out.append(inst)
            b.instructions = out
# ---------------------------------------------------------------------------

N_CORES = 8
NUM_GRAPHS = 2048
SEGS_PER_CORE = NUM_GRAPHS // N_CORES  # 256
N_BLOCKS = 16  # 128-segment blocks; 2 per core (= regions)
D = 256
G = 8  # nodes per group (segment starts padded to multiples of G)
ST_NODES = 1024  # nodes per super-tile (8 subtiles x 128)
K_SUB = 8
LN_EPS = 1e-5
NEG_SLOPE = 0.01

_F16 = mybir.dt.float16
_F8 = mybir.dt.float8e4
_F8E3 = mybir.dt.float8e3
_F32 = mybir.dt.float32
_ALU = mybir.AluOpType

# Test/debug hooks: set PROFILE=True before calling kernel() to request an
# NTFF trace; the BassKernelResults lands in LAST_RESULT.
PROFILE = False
PROFILE_DIR = None
LAST_RESULT = None


def _build_program(r_st: int) -> bass.Bass:
    """r_st: super-tiles per 128-segment region (2 regions per core)."""
    NT = 2 * r_st  # super-tiles per core

    nc = bass.Bass("TRN2", debug=False)
    # features ride split precision: half the columns fp8e3m4, half fp16
    # (384B/node vs 512; measured end-to-end rel err 1.26e-2 vs the 2e-2 gate)
    feat8 = nc.dram_tensor("feat8", [NT * 128, K_SUB * 128], _F8E3, kind="ExternalInput").ap()
    feat16 = nc.dram_tensor("feat16", [NT * 128, K_SUB * 128], _F16, kind="ExternalInput").ap()
    s_d = nc.dram_tensor("smat", [128, K_SUB * 128], _F16, kind="ExternalInput").ap()
    s8_d = nc.dram_tensor("smat8", [128, K_SUB * 128], _F8E3, kind="ExternalInput").ap()
    oh_d = nc.dram_tensor("oh", [128, NT * 128], _F8, kind="ExternalInput").ap()
    ident_d = nc.dram_tensor("ident", [128, 128], _F32, kind="ExternalInput").ap()
    w1aug_d = nc.dram_tensor("w1aug", [D + 1, 128], _F32, kind="ExternalInput").ap()
    pvec_d = nc.dram_tensor("pvec", [1, 385], _F32, kind="ExternalInput").ap()
    rec_d = nc.dram_tensor("rec", [128, 2], _F32, kind="ExternalInput").ap()
    out_d = nc.dram_tensor("out", [2, 128], _F32, kind="ExternalOutput").ap()

    SKEW = 3  # stage-2 trails stage-1 by this many super-tiles on the PE queue
    HEAD_SKEW = 2  # region head trails its last stage-2 matmul

    with tile.TileContext(nc) as tc, ExitStack() as ctx:
        cpool = ctx.enter_context(tc.tile_pool(name="consts", bufs=1))
        fpool = ctx.enter_context(tc.tile_pool(name="feat", bufs=8))
        f8pool = ctx.enter_context(tc.tile_pool(name="feat8", bufs=8))
        gpool = ctx.enter_context(tc.tile_pool(name="gp", bufs=2, space="PSUM"))
        gspool = ctx.enter_context(tc.tile_pool(name="gs", bufs=6))
        acc = ctx.enter_context(tc.tile_pool(name="acc", bufs=1, space="PSUM"))
        ppool = ctx.enter_context(tc.tile_pool(name="pw", bufs=1, space="PSUM"))
        spool = ctx.enter_context(tc.tile_pool(name="small", bufs=2))

        # tiny uploads first on the sync queue: its completion counter gates
        # the first PE work, so nothing fat may precede these.
        pv = cpool.tile([1, 385], _F32, tag="pv")
        nc.sync.dma_start(out=pv[:], in_=pvec_d[:])
        rec_t = cpool.tile([128, 2], _F32, tag="rec")
        nc.sync.dma_start(out=rec_t[:], in_=rec_d[:])
        # smat + the fat one-hot ride the scalar engine's DMA queue (it issues
        # no other DMAs, so stage-1 releases on just the smat completion)
        s_t = cpool.tile([128, K_SUB * 128], _F16, tag="smat")
        nc.scalar.dma_start(out=s_t[:], in_=s_d[:])
        s8_t = cpool.tile([128, K_SUB * 128], _F8E3, tag="smat8")
        nc.scalar.dma_start(out=s8_t[:], in_=s8_d[:])
        oh_t = cpool.tile([128, NT * 128], _F8, tag="oh")
        nc.scalar.dma_start(out=oh_t[:], in_=oh_d[:])
        ident_t = cpool.tile([128, 128], _F32, tag="ident")
        w1a = cpool.tile([128, 128], _F32, tag="w1a")
        w1b = cpool.tile([128, 128], _F32, tag="w1b")
        w1c = cpool.tile([1, 128], _F32, tag="w1c")
        ones_row = cpool.tile([1, 256], _F32, tag="ones")
        nc.vector.memset(ones_row[:], 1.0)
        epsc = cpool.tile([128, 1], _F32, tag="epsc")
        nc.vector.memset(epsc[:], LN_EPS)
        bc = cpool.tile([128, 385], _F32, tag="bcs")

        sums = [acc.tile([128, D], _F32, tag=f"sum{r}", name=f"sum{r}") for r in range(2)]
        ptT = [spool.tile([128, 256], _F32, tag=f"ptT{fb}", name=f"ptT{fb}") for fb in range(2)]

        def emit_head_transposes(r):
            # sums[r] already holds pooled means (1/count folded into oh)
            pooled = spool.tile([128, 256], _F32, tag=f"pooled{r}", name=f"pooled{r}")
            nc.scalar.mul(pooled[:], sums[r][:], rec_t[:, r : r + 1])
            for fb in range(2):
                tp = ppool.tile([128, 128], _F32, tag="tp")
                nc.tensor.transpose(
                    out=tp[:], in_=pooled[:, fb * 128 : (fb + 1) * 128],
                    identity=ident_t[:],
                )
                nc.scalar.copy(ptT[fb][:, r * 128 : (r + 1) * 128], tp[:])

        def emit_head(m):
            # h = pooled @ W1 + b1; LayerNorm; LeakyReLU; @ W2 + b2
            msl = slice(m * 128, (m + 1) * 128)
            h_ps = ppool.tile([128, 128], _F32, tag="h")
            nc.tensor.matmul(
                out=h_ps[:], lhsT=ptT[0][:, msl], rhs=w1a[:], start=True, stop=False
            )
            nc.tensor.matmul(
                out=h_ps[:], lhsT=ptT[1][:, msl], rhs=w1b[:], start=False, stop=False
            )
            nc.tensor.matmul(
                out=h_ps[:], lhsT=ones_row[:, msl], rhs=w1c[:], start=False, stop=True
            )

            musum = spool.tile([128, 1], _F32, tag="musum")
            nc.vector.tensor_reduce(
                out=musum[:], in_=h_ps[:], axis=mybir.AxisListType.X, op=_ALU.add
            )
            mu = spool.tile([128, 1], _F32, tag="mu")
            nc.vector.tensor_scalar(
                out=mu[:], in0=musum[:], scalar1=1.0 / 128, scalar2=None, op0=_ALU.mult
            )
            hc = spool.tile([128, 128], _F32, tag="hc")
            nc.vector.tensor_scalar(
                out=hc[:], in0=h_ps[:], scalar1=mu[:], scalar2=None, op0=_ALU.subtract
            )
            sq = spool.tile([128, 128], _F32, tag="sq")
            ssq = spool.tile([128, 1], _F32, tag="ssq")
            nc.vector.scalar_tensor_tensor(
                out=sq[:], in0=hc[:], scalar=1.0, in1=hc[:],
                op0=_ALU.mult, op1=_ALU.mult, accum_out=ssq[:],
            )
            std = spool.tile([128, 1], _F32, tag="std")
            nc.scalar.activation(
                std[:], ssq[:], mybir.ActivationFunctionType.Sqrt,
                bias=epsc[:], scale=1.0 / 128,
            )
            rstd = spool.tile([128, 1], _F32, tag="rstd")
            nc.vector.reciprocal(rstd[:], std[:])
            y = spool.tile([128, 128], _F32, tag="y")
            nc.vector.scalar_tensor_tensor(
                out=y[:], in0=hc[:], scalar=rstd[:], in1=bc[:, 0:128],
                op0=_ALU.mult, op1=_ALU.mult,
            )
            y2 = spool.tile([128, 128], _F32, tag="y2")
            nc.vector.tensor_tensor(out=y2[:], in0=y[:], in1=bc[:, 128:256],
                                    op=_ALU.add)
            yl = spool.tile([128, 128], _F32, tag="yl")
            nc.vector.scalar_tensor_tensor(
                out=yl[:], in0=y2[:], scalar=NEG_SLOPE, in1=y2[:],
                op0=_ALU.mult, op1=_ALU.max,
            )
            prod = spool.tile([128, 128], _F32, tag="prod")
            oc = spool.tile([128, 1], _F32, tag="oc")
            nc.vector.scalar_tensor_tensor(
                out=prod[:], in0=yl[:], scalar=1.0, in1=bc[:, 256:384],
                op0=_ALU.mult, op1=_ALU.mult, accum_out=oc[:],
            )
            ofin = spool.tile([128, 1], _F32, tag="ofin")
            nc.vector.tensor_scalar(
                out=ofin[:], in0=oc[:], scalar1=bc[:, 384:385], scalar2=None,
                op0=_ALU.add,
            )
            nc.sync.dma_start(out=out_d[m, :], in_=ofin[:])

        # ---- main stream: two-stage segment means ----
        gs_tiles = [None] * NT
        ft = None
        for idx in range(NT + SKEW):
            if idx < NT:
                st = idx
                ft8 = f8pool.tile([128, K_SUB * 128], _F8E3, tag="ft8")
                ft16 = fpool.tile([128, K_SUB * 128], _F16, tag="ft")
                dma_eng = nc.sync if st % 2 == 0 else nc.gpsimd
                dma_eng.dma_start(out=ft8[:], in_=feat8[st * 128 : (st + 1) * 128, :])
                dma_eng.dma_start(out=ft16[:], in_=feat16[st * 128 : (st + 1) * 128, :])
                # the two dtype groups must NOT interleave within one PSUM
                # bank: alternating fp8/fp16 matmuls into the same bank
                # corrupts the fp8 accumulation (probed on HW)
                gp = gpool.tile([128, D], _F32, tag="gp")
                for k in range(K_SUB):
                    ksl = slice(k * 128, (k + 1) * 128)
                    nc.tensor.matmul(
                        out=gp[:, 0:128],
                        lhsT=s8_t[:, ksl],
                        rhs=ft8[:, ksl],
                        start=(k == 0),
                        stop=(k == K_SUB - 1),
                    )
                for k in range(K_SUB):
                    ksl = slice(k * 128, (k + 1) * 128)
                    nc.tensor.matmul(
                        out=gp[:, 128:256],
                        lhsT=s_t[:, ksl],
                        rhs=ft16[:, ksl],
                        start=(k == 0),
                        stop=(k == K_SUB - 1),
                    )
                gs = gspool.tile([128, D], _F16, tag="gs")
                nc.scalar.copy(gs[:], gp[:])
                gs_tiles[st] = gs
            if idx == 6:
                # head-only constants: uploaded behind the first feature
                # chunks so they never gate the stream
                nc.sync.dma_start(out=ident_t[:], in_=ident_d[:])
                nc.sync.dma_start(out=w1a[:], in_=w1aug_d[0:128, :])
                nc.sync.dma_start(out=w1b[:], in_=w1aug_d[128:256, :])
                nc.sync.dma_start(out=w1c[:], in_=w1aug_d[256:257, :])
                # broadcast [gamma | beta | W2 | b2] to all 128 partitions —
                # emitted here so it doesn't sit at the head of the PE queue
                bc_ps = ppool.tile([128, 385], _F32, tag="bc")
                nc.tensor.matmul(
                    out=bc_ps[:], lhsT=ones_row[:, 0:128], rhs=pv[:],
                    start=True, stop=True,
                )
                nc.scalar.copy(bc[:], bc_ps[:])
            if idx >= SKEW:
                # stage 2, trailing so the in-order PE queue never stalls on
                # the activation-engine PSUM->SBUF copy or the oh upload
                st2 = idx - SKEW
                r2, stl = divmod(st2, r_st)
                nc.tensor.matmul(
                    out=sums[r2][:],
                    lhsT=oh_t[:, st2 * 128 : (st2 + 1) * 128],
                    rhs=gs_tiles[st2][:],
                    start=(stl == 0),
                    stop=(stl == r_st - 1),
                )
                gs_tiles[st2] = None
            # region-0 head hides under region 1's stream
            if idx == r_st - 1 + SKEW + HEAD_SKEW:
                emit_head_transposes(0)
            if idx == r_st - 1 + SKEW + 2 * HEAD_SKEW:
                emit_head(0)

        emit_head_transposes(1)
        emit_head(1)

    _split_excess_waits(nc)
    return nc


def _prep_inputs(features, batch):
    """Group-aligned padded layout + per-core arrays.

    Returns (feat_cores [8, NT*128, 2048] f16, oh_cores [8, 128, NT*128] f16,
    r_st).
    """
    feats16 = np.asarray(features).astype(np.float16)
    seg = np.asarray(batch).astype(np.int64)
    n = seg.shape[0]
    counts = np.bincount(seg, minlength=NUM_GRAPHS)
    bnd = np.zeros(NUM_GRAPHS + 1, np.int64)
    bnd[1:] = np.cumsum(counts)

    # each segment starts at a multiple of G inside its 128-segment block
    pad_counts = ((counts + G - 1) // G) * G
    block_of_seg = np.arange(NUM_GRAPHS) // 128
    # per-block padded totals and r_st (shared by all cores: one SPMD program)
    blk_tot = np.zeros(N_BLOCKS, np.int64)
    np.add.at(blk_tot, block_of_seg, pad_counts)
    r_st = int(np.max((blk_tot + ST_NODES - 1) // ST_NODES))
    cap = r_st * ST_NODES  # padded node slots per block

    # start slot of each segment inside its block
    cum = np.cumsum(pad_counts)
    seg_start = cum - pad_counts
    blk_base = np.zeros(NUM_GRAPHS, np.int64)
    first_seg = np.arange(0, NUM_GRAPHS, 128)
    blk_base[first_seg] = seg_start[first_seg]
    blk_base = np.maximum.accumulate(blk_base)  # block-start offset per seg
    seg_start_local = seg_start - blk_base

    # scatter nodes into the padded [16, cap] layout
    rank = np.arange(n) - bnd[seg]
    dest = block_of_seg[seg] * cap + seg_start_local[seg] + rank
    fpad = np.zeros((N_BLOCKS * cap, D), np.float16)
    fpad[dest] = feats16
    segpad = np.full(N_BLOCKS * cap, -1, np.int64)
    segpad[dest] = seg

    # permute to the on-chip super-tile layout: slot (st*1024 + k*128 + p)
    # lands at row st*128+p, cols k*128..  ->  [blk, r_st, 128, 8, 128] per half
    def _permute(arr):
        return np.ascontiguousarray(
            arr.reshape(N_BLOCKS, r_st, K_SUB, 128, 128)
            .transpose(0, 1, 3, 2, 4)
            .reshape(N_CORES, 2 * r_st * 128, K_SUB * 128)
        )

    feat8_cores = _permute(fpad[:, 0:128].astype(ml_dtypes.float8_e3m4))
    feat16_cores = _permute(fpad[:, 128:256])

    # group segment ids: group g of block b = slots [8g, 8g+8) (uniform by
    # construction; first slot of a non-empty group is always a real node)
    gseg = segpad[::G].reshape(N_BLOCKS, r_st * 128)
    gseg_local = gseg - 128 * np.arange(N_BLOCKS)[:, None]  # pad rows -> <0
    # 0/1 one-hot [blk, st*128+g, s] (exact in fp8e4m3);
    # transpose to SBUF layout [blk, g(128), st, s]
    oh = (gseg_local[:, :, None] == np.arange(128)[None, None, :]).astype(
        ml_dtypes.float8_e4m3
    )
    oh = (
        oh.reshape(N_BLOCKS, r_st, 128, 128)
        .transpose(0, 2, 1, 3)
        .reshape(N_BLOCKS, 128, r_st * 128)
    )
    # core i holds blocks 2i (region 0) and 2i+1 (region 1) side by side
    oh_cores = np.ascontiguousarray(
        oh.reshape(N_CORES, 2, 128, r_st * 128)
        .transpose(0, 2, 1, 3)
        .reshape(N_CORES, 128, 2 * r_st * 128)
    )

    rec = (1.0 / np.maximum(counts, 1)).astype(np.float32)
    rec_cores = np.ascontiguousarray(
        rec.reshape(N_CORES, 2, 128).transpose(0, 2, 1)
    )
    return feat8_cores, feat16_cores, oh_cores, rec_cores, r_st


def kernel(features, batch, W1, b1, gamma, beta, W2, b2):
    feat8_cores, feat16_cores, oh_cores, rec_cores, r_st = _prep_inputs(features, batch)

    # fixed stage-1 stationaries: S[p, k, q] = 1 iff q == 16k + p//8
    p = np.arange(128)
    smat = np.zeros((128, K_SUB, 128), np.float16)
    for k in range(K_SUB):
        smat[p, k, 16 * k + p // G] = 1.0
    smat = smat.reshape(128, K_SUB * 128)

    ident = np.eye(128, dtype=np.float32)
    w1aug = np.concatenate(
        [np.asarray(W1, np.float32), np.asarray(b1, np.float32)[None, :]], axis=0
    )
    pvec = np.concatenate(
        [
            np.asarray(gamma, np.float32).ravel(),
            np.asarray(beta, np.float32).ravel(),
            np.asarray(W2, np.float32).ravel(),
            np.asarray(b2, np.float32).ravel(),
        ]
    )[None, :]

    nc = _build_program(r_st)
    in_maps = [
        {
            "feat8": feat8_cores[i],
            "feat16": feat16_cores[i],
            "smat": smat,
            "smat8": smat.astype(ml_dtypes.float8_e3m4),
            "oh": oh_cores[i],
            "ident": ident,
            "w1aug": w1aug,
            "pvec": pvec,
            "rec": rec_cores[i],
        }
        for i in range(N_CORES)
    ]
    res = run_bass_kernel_spmd(
        nc, in_maps, list(range(N_CORES)), trace=PROFILE, tmpdir=PROFILE_DIR
    )
    global LAST_RESULT
    LAST_RESULT = res
    out = np.concatenate(
        [res.results[i]["out"].reshape(SEGS_PER_CORE) for i in range(N_CORES)]
    )
    return out.reshape(NUM_GRAPHS, 1).astype(np.float32)


# revision 22
# speedup vs baseline: 1.0618x; 1.0069x over previous
"""Trainium2 Bass kernel for nn_Classifier (segment mean-pool + tiny MLP head).

Pipeline (matches the jax reference):
  pooled[g] = mean of features over nodes with batch id g   (2048 graphs)
  out = LeakyReLU(LayerNorm(pooled @ W1 + b1)) @ W2 + b2    -> [2048, 1]

Sharding: batch ids are sorted, so nodes split across the 8 cores at
segment-block boundaries — core i owns graphs [256i, 256i+256) (two
128-segment regions) and exactly the nodes belonging to them. Segment sums
are disjoint per core, so no collective is needed; the host concatenates the
8 per-core [256]-sized outputs.

Per-core compute is a two-stage PE-only reduction with no runtime one-hot
generation (the vector/gpsimd is_equal one-hots were the measured bottleneck
of the previous version):
  stage 1: each 1024-node super-tile is summed into 128 group-sums (groups of
           8 consecutive nodes) via 8 matmuls whose stationaries are FIXED
           0/1 matrices uploaded once (S_k maps subtile k's 128 nodes to
           groups 16k..16k+16).
  stage 2: one matmul per super-tile scatters the 128 group-sums into the
           region's 128 segment rows using a host-built one-hot (the host
           knows every segment boundary), accumulating in PSUM. The one-hot
           carries 1/count instead of 1, so PSUM accumulates segment MEANS
           directly and no per-partition divide is needed.
The host pads each segment to start on a group (8-node) boundary (~1.5%
zero-pad), so every group belongs to exactly one segment. Features travel as
fp16 (halves HBM traffic vs fp32; verified end-to-end rel err ~4e-4 vs the
2e-2 gate). Segment counts come from a host bincount.

Scheduling notes: the 2MB one-hot upload goes on the vector engine's DMA
queue so the sync queue's completion counter (which gates the first PE work)
only covers the tiny pvec/smat uploads and the first feature chunks. Stage-2
matmuls are emitted 3 super-tiles behind stage 1 so the in-order PE queue
never stalls on the activation-engine PSUM->SBUF copies. Each region's MLP
head is emitted as soon as that region's stage-2 accumulation stops, hiding
region 0's head under region 1's feature stream.
"""

from contextlib import ExitStack

import ml_dtypes
import numpy as np

import concourse.bass as bass
import concourse.mybir as mybir
import concourse.tile as tile
from concourse.bass_utils import run_bass_kernel_spmd

# ---------------------------------------------------------------------------
# Workaround: this walrus build rejects instructions carrying more than one
# semaphore wait ("Too many sync wait commands"), but Tile's semaphore
# assignment freely attaches several. After the TileContext has lowered the
# program, split any excess waits onto same-engine nops inserted right before
# the instruction (semantics are identical: all waits are monotonic and must
# hold before the instruction issues).
_MAX_WAITS = 1


def _split_excess_waits(nc: "bass.Bass", max_waits: int = _MAX_WAITS) -> None:
    ctr = 0
    for f in nc.m.functions:
        for b in f.blocks:
            out = []
            for inst in b.instructions:
                si = inst.sync_info
                waits = list(si.on_wait) if (si is not None and si.on_wait) else []
                if len(waits) > max_waits:
                    keep = waits[-max_waits:]
                    extra = waits[:-max_waits]
                    # On the PE queue the carrier must be a DRAIN: silicon
                    # promotes waitless LDWEIGHTS past in-flight work, so a
                    # plain nop's wait can be bypassed (walrus attaches a
                    # matmul's waits to its LDWEIGHTS — stripping them onto a
                    # nop re-opens that race). A drain fully serializes.
                    is_pe = inst.engine == mybir.EngineType.PE
                    for i in range(0, len(extra), max_waits):
                        ctr += 1
                        if is_pe:
                            nop = mybir.InstDrain(
                                name=f"waitsplit_drain_{ctr}", ins=[], outs=[],
                                engine=inst.engine,
                            )
                        else:
                            nop = mybir.InstNoOp(
                                name=f"waitsplit_nop_{ctr}", ins=[], outs=[],
                                engine=inst.engine,
                            )
                        nop.sync_info = mybir.SyncInfo(
                            on_wait=extra[i : i + max_waits], on_update=[]
                        )
                        nc.register_instruction(nop)
                        out.append(nop)
                    inst.sync_info = mybir.SyncInfo(
                        on_wait=keep, on_update=list(si.on_update or [])
                    )
                out.append(inst)
            b.instructions = out
# ---------------------------------------------------------------------------

N_CORES = 8
NUM_GRAPHS = 2048
SEGS_PER_CORE = NUM_GRAPHS // N_CORES  # 256
N_BLOCKS = 16  # 128-segment blocks; 2 per core (= regions)
D = 256
G = 8  # nodes per group (segment starts padded to multiples of G)
ST_NODES = 1024  # nodes per super-tile (8 subtiles x 128)
K_SUB = 8
LN_EPS = 1e-5
NEG_SLOPE = 0.01

_F16 = mybir.dt.float16
_F8 = mybir.dt.float8e4
_F8E3 = mybir.dt.float8e3
_F32 = mybir.dt.float32
_ALU = mybir.AluOpType

# Test/debug hooks: set PROFILE=True before calling kernel() to request an
# NTFF trace; the BassKernelResults lands in LAST_RESULT.
PROFILE = False
PROFILE_DIR = None
LAST_RESULT = None


def _build_program(r_st: int) -> bass.Bass:
    """r_st: super-tiles per 128-segment region (2 regions per core)."""
    NT = 2 * r_st  # super-tiles per core

    nc = bass.Bass("TRN2", debug=False)
    # features ride split precision: half the columns fp8e3m4, half fp16
    # (384B/node vs 512; measured end-to-end rel err 1.26e-2 vs the 2e-2 gate)
    feat8 = nc.dram_tensor("feat8", [NT * 128, K_SUB * 128], _F8E3, kind="ExternalInput").ap()
    feat16 = nc.dram_tensor("feat16", [NT * 128, K_SUB * 128], _F16, kind="ExternalInput").ap()
    s_d = nc.dram_tensor("smat", [128, K_SUB * 128], _F16, kind="ExternalInput").ap()
    s8_d = nc.dram_tensor("smat8", [128, K_SUB * 128], _F8E3, kind="ExternalInput").ap()
    oh_d = nc.dram_tensor("oh", [128, NT * 128], _F8, kind="ExternalInput").ap()
    ident_d = nc.dram_tensor("ident", [128, 128], _F32, kind="ExternalInput").ap()
    w1aug_d = nc.dram_tensor("w1aug", [D + 1, 128], _F32, kind="ExternalInput").ap()
    pvec_d = nc.dram_tensor("pvec", [1, 385], _F32, kind="ExternalInput").ap()
    rec_d = nc.dram_tensor("rec", [128, 2], _F32, kind="ExternalInput").ap()
    out_d = nc.dram_tensor("out", [2, 128], _F32, kind="ExternalOutput").ap()

    SKEW = 3  # stage-2 trails stage-1 by this many super-tiles on the PE queue
    HEAD_SKEW = 2  # region head trails its last stage-2 matmul

    with tile.TileContext(nc) as tc, ExitStack() as ctx:
        cpool = ctx.enter_context(tc.tile_pool(name="consts", bufs=1))
        fpool = ctx.enter_context(tc.tile_pool(name="feat", bufs=8))
        f8pool = ctx.enter_context(tc.tile_pool(name="feat8", bufs=8))
        gpool = ctx.enter_context(tc.tile_pool(name="gp", bufs=2, space="PSUM"))
        gspool = ctx.enter_context(tc.tile_pool(name="gs", bufs=6))
        acc = ctx.enter_context(tc.tile_pool(name="acc", bufs=1, space="PSUM"))
        ppool = ctx.enter_context(tc.tile_pool(name="pw", bufs=1, space="PSUM"))
        spool = ctx.enter_context(tc.tile_pool(name="small", bufs=2))

        # tiny uploads first on the sync queue: its completion counter gates
        # the first PE work, so nothing fat may precede these.
        pv = cpool.tile([1, 385], _F32, tag="pv")
        nc.sync.dma_start(out=pv[:], in_=pvec_d[:])
        rec_t = cpool.tile([128, 2], _F32, tag="rec")
        nc.sync.dma_start(out=rec_t[:], in_=rec_d[:])
        # smat + the fat one-hot ride the scalar engine's DMA queue (it issues
        # no other DMAs, so stage-1 releases on just the smat completion)
        s_t = cpool.tile([128, K_SUB * 128], _F16, tag="smat")
        nc.scalar.dma_start(out=s_t[:], in_=s_d[:])
        s8_t = cpool.tile([128, K_SUB * 128], _F8E3, tag="smat8")
        nc.scalar.dma_start(out=s8_t[:], in_=s8_d[:])
        oh_t = cpool.tile([128, NT * 128], _F8, tag="oh")
        nc.scalar.dma_start(out=oh_t[:], in_=oh_d[:])
        ident_t = cpool.tile([128, 128], _F32, tag="ident")
        w1a = cpool.tile([128, 128], _F32, tag="w1a")
        w1b = cpool.tile([128, 128], _F32, tag="w1b")
        w1c = cpool.tile([1, 128], _F32, tag="w1c")
        ones_row = cpool.tile([1, 256], _F32, tag="ones")
        nc.vector.memset(ones_row[:], 1.0)
        epsc = cpool.tile([128, 1], _F32, tag="epsc")
        nc.vector.memset(epsc[:], LN_EPS)
        bc = cpool.tile([128, 385], _F32, tag="bcs")

        sums = [acc.tile([128, D], _F32, tag=f"sum{r}", name=f"sum{r}") for r in range(2)]
        ptT = [spool.tile([128, 256], _F32, tag=f"ptT{fb}", name=f"ptT{fb}") for fb in range(2)]

        def emit_head_transposes(r):
            # sums[r] already holds pooled means (1/count folded into oh)
            pooled = spool.tile([128, 256], _F32, tag=f"pooled{r}", name=f"pooled{r}")
            nc.scalar.mul(pooled[:], sums[r][:], rec_t[:, r : r + 1])
            for fb in range(2):
                tp = ppool.tile([128, 128], _F32, tag="tp")
                nc.tensor.transpose(
                    out=tp[:], in_=pooled[:, fb * 128 : (fb + 1) * 128],
                    identity=ident_t[:],
                )
                nc.scalar.copy(ptT[fb][:, r * 128 : (r + 1) * 128], tp[:])

        def emit_head(m):
            # h = pooled @ W1 + b1; LayerNorm; LeakyReLU; @ W2 + b2
            msl = slice(m * 128, (m + 1) * 128)
            h_ps = ppool.tile([128, 128], _F32, tag="h")
            nc.tensor.matmul(
                out=h_ps[:], lhsT=ptT[0][:, msl], rhs=w1a[:], start=True, stop=False
            )
            nc.tensor.matmul(
                out=h_ps[:], lhsT=ptT[1][:, msl], rhs=w1b[:], start=False, stop=False
            )
            nc.tensor.matmul(
                out=h_ps[:], lhsT=ones_row[:, msl], rhs=w1c[:], start=False, stop=True
            )

            musum = spool.tile([128, 1], _F32, tag="musum")
            nc.vector.tensor_reduce(
                out=musum[:], in_=h_ps[:], axis=mybir.AxisListType.X, op=_ALU.add
            )
            mu = spool.tile([128, 1], _F32, tag="mu")
            nc.vector.tensor_scalar(
                out=mu[:], in0=musum[:], scalar1=1.0 / 128, scalar2=None, op0=_ALU.mult
            )
            hc = spool.tile([128, 128], _F32, tag="hc")
            nc.vector.tensor_scalar(
                out=hc[:], in0=h_ps[:], scalar1=mu[:], scalar2=None, op0=_ALU.subtract
            )
            sq = spool.tile([128, 128], _F32, tag="sq")
            ssq = spool.tile([128, 1], _F32, tag="ssq")
            nc.vector.scalar_tensor_tensor(
                out=sq[:], in0=hc[:], scalar=1.0, in1=hc[:],
                op0=_ALU.mult, op1=_ALU.mult, accum_out=ssq[:],
            )
            std = spool.tile([128, 1], _F32, tag="std")
            nc.scalar.activation(
                std[:], ssq[:], mybir.ActivationFunctionType.Sqrt,
                bias=epsc[:], scale=1.0 / 128,
            )
            rstd = spool.tile([128, 1], _F32, tag="rstd")
            nc.vector.reciprocal(rstd[:], std[:])
            y = spool.tile([128, 128], _F32, tag="y")
            nc.vector.scalar_tensor_tensor(
                out=y[:], in0=hc[:], scalar=rstd[:], in1=bc[:, 0:128],
                op0=_ALU.mult, op1=_ALU.mult,
            )
            y2 = spool.tile([128, 128], _F32, tag="y2")
            nc.vector.tensor_tensor(out=y2[:], in0=y[:], in1=bc[:, 128:256],
                                    op=_ALU.add)
            yl = spool.tile([128, 128], _F32, tag="yl")
            nc.vector.scalar_tensor_tensor(
                out=yl[:], in0=y2[:], scalar=NEG_SLOPE, in1=y2[:],
                op0=_ALU.mult, op1=_ALU.max,
            )
            prod = spool.tile([128, 128], _F32, tag="prod")
            oc = spool.tile([128, 1], _F32, tag="oc")
            nc.vector.scalar_tensor_tensor(
                out=prod[:], in0=yl[:], scalar=1.0, in1=bc[:, 256:384],
                op0=_ALU.mult, op1=_ALU.mult, accum_out=oc[:],
            )
            ofin = spool.tile([128, 1], _F32, tag="ofin")
            nc.vector.tensor_scalar(
                out=ofin[:], in0=oc[:], scalar1=bc[:, 384:385], scalar2=None,
                op0=_ALU.add,
            )
            nc.sync.dma_start(out=out_d[m, :], in_=ofin[:])

        # ---- main stream: two-stage segment means ----
        gs_tiles = [None] * NT
        ft = None
        for idx in range(NT + SKEW):
            if idx < NT:
                st = idx
                # one issue per engine per super-tile: two per engine would
                # throttle the stream on DIRECT2D dispatch time
                ft8 = f8pool.tile([128, K_SUB * 128], _F8E3, tag="ft8")
                ft16 = fpool.tile([128, K_SUB * 128], _F16, tag="ft")
                nc.sync.dma_start(out=ft8[:], in_=feat8[st * 128 : (st + 1) * 128, :])
                nc.gpsimd.dma_start(out=ft16[:], in_=feat16[st * 128 : (st + 1) * 128, :])
                # the two dtype groups must NOT interleave within one PSUM
                # bank: alternating fp8/fp16 matmuls into the same bank
                # corrupts the fp8 accumulation (probed on HW)
                gp = gpool.tile([128, D], _F32, tag="gp")
                for k in range(K_SUB):
                    ksl = slice(k * 128, (k + 1) * 128)
                    nc.tensor.matmul(
                        out=gp[:, 0:128],
                        lhsT=s8_t[:, ksl],
                        rhs=ft8[:, ksl],
                        start=(k == 0),
                        stop=(k == K_SUB - 1),
                    )
                for k in range(K_SUB):
                    ksl = slice(k * 128, (k + 1) * 128)
                    nc.tensor.matmul(
                        out=gp[:, 128:256],
                        lhsT=s_t[:, ksl],
                        rhs=ft16[:, ksl],
                        start=(k == 0),
                        stop=(k == K_SUB - 1),
                    )
                gs = gspool.tile([128, D], _F16, tag="gs")
                nc.scalar.copy(gs[:], gp[:])
                gs_tiles[st] = gs
            if idx == 6:
                # head-only constants: uploaded behind the first feature
                # chunks so they never gate the stream
                nc.sync.dma_start(out=ident_t[:], in_=ident_d[:])
                nc.sync.dma_start(out=w1a[:], in_=w1aug_d[0:128, :])
                nc.sync.dma_start(out=w1b[:], in_=w1aug_d[128:256, :])
                nc.sync.dma_start(out=w1c[:], in_=w1aug_d[256:257, :])
                # broadcast [gamma | beta | W2 | b2] to all 128 partitions —
                # emitted here so it doesn't sit at the head of the PE queue
                bc_ps = ppool.tile([128, 385], _F32, tag="bc")
                nc.tensor.matmul(
                    out=bc_ps[:], lhsT=ones_row[:, 0:128], rhs=pv[:],
                    start=True, stop=True,
                )
                nc.scalar.copy(bc[:], bc_ps[:])
            if idx >= SKEW:
                # stage 2, trailing so the in-order PE queue never stalls on
                # the activation-engine PSUM->SBUF copy or the oh upload
                st2 = idx - SKEW
                r2, stl = divmod(st2, r_st)
                nc.tensor.matmul(
                    out=sums[r2][:],
                    lhsT=oh_t[:, st2 * 128 : (st2 + 1) * 128],
                    rhs=gs_tiles[st2][:],
                    start=(stl == 0),
                    stop=(stl == r_st - 1),
                )
                gs_tiles[st2] = None
            # region-0 head hides under region 1's stream
            if idx == r_st - 1 + SKEW + HEAD_SKEW:
                emit_head_transposes(0)
            if idx == r_st - 1 + SKEW + 2 * HEAD_SKEW:
                emit_head(0)

        emit_head_transposes(1)
        emit_head(1)

    _split_excess_waits(nc)
    return nc


def _prep_inputs(features, batch):
    """Group-aligned padded layout + per-core arrays.

    Returns (feat_cores [8, NT*128, 2048] f16, oh_cores [8, 128, NT*128] f16,
    r_st).
    """
    feats16 = np.asarray(features).astype(np.float16)
    seg = np.asarray(batch).astype(np.int64)
    n = seg.shape[0]
    counts = np.bincount(seg, minlength=NUM_GRAPHS)
    bnd = np.zeros(NUM_GRAPHS + 1, np.int64)
    bnd[1:] = np.cumsum(counts)

    # each segment starts at a multiple of G inside its 128-segment block
    pad_counts = ((counts + G - 1) // G) * G
    block_of_seg = np.arange(NUM_GRAPHS) // 128
    # per-block padded totals and r_st (shared by all cores: one SPMD program)
    blk_tot = np.zeros(N_BLOCKS, np.int64)
    np.add.at(blk_tot, block_of_seg, pad_counts)
    r_st = int(np.max((blk_tot + ST_NODES - 1) // ST_NODES))
    cap = r_st * ST_NODES  # padded node slots per block

    # start slot of each segment inside its block
    cum = np.cumsum(pad_counts)
    seg_start = cum - pad_counts
    blk_base = np.zeros(NUM_GRAPHS, np.int64)
    first_seg = np.arange(0, NUM_GRAPHS, 128)
    blk_base[first_seg] = seg_start[first_seg]
    blk_base = np.maximum.accumulate(blk_base)  # block-start offset per seg
    seg_start_local = seg_start - blk_base

    # scatter nodes into the padded [16, cap] layout
    rank = np.arange(n) - bnd[seg]
    dest = block_of_seg[seg] * cap + seg_start_local[seg] + rank
    fpad = np.zeros((N_BLOCKS * cap, D), np.float16)
    fpad[dest] = feats16
    segpad = np.full(N_BLOCKS * cap, -1, np.int64)
    segpad[dest] = seg

    # permute to the on-chip super-tile layout: slot (st*1024 + k*128 + p)
    # lands at row st*128+p, cols k*128..  ->  [blk, r_st, 128, 8, 128] per half
    def _permute(arr):
        return np.ascontiguousarray(
            arr.reshape(N_BLOCKS, r_st, K_SUB, 128, 128)
            .transpose(0, 1, 3, 2, 4)
            .reshape(N_CORES, 2 * r_st * 128, K_SUB * 128)
        )

    feat8_cores = _permute(fpad[:, 0:128].astype(ml_dtypes.float8_e3m4))
    feat16_cores = _permute(fpad[:, 128:256])

    # group segment ids: group g of block b = slots [8g, 8g+8) (uniform by
    # construction; first slot of a non-empty group is always a real node)
    gseg = segpad[::G].reshape(N_BLOCKS, r_st * 128)
    gseg_local = gseg - 128 * np.arange(N_BLOCKS)[:, None]  # pad rows -> <0
    # 0/1 one-hot [blk, st*128+g, s] (exact in fp8e4m3);
    # transpose to SBUF layout [blk, g(128), st, s]
    oh = (gseg_local[:, :, None] == np.arange(128)[None, None, :]).astype(
        ml_dtypes.float8_e4m3
    )
    oh = (
        oh.reshape(N_BLOCKS, r_st, 128, 128)
        .transpose(0, 2, 1, 3)
        .reshape(N_BLOCKS, 128, r_st * 128)
    )
    # core i holds blocks 2i (region 0) and 2i+1 (region 1) side by side
    oh_cores = np.ascontiguousarray(
        oh.reshape(N_CORES, 2, 128, r_st * 128)
        .transpose(0, 2, 1, 3)
        .reshape(N_CORES, 128, 2 * r_st * 128)
    )

    rec = (1.0 / np.maximum(counts, 1)).astype(np.float32)
    rec_cores = np.ascontiguousarray(
        rec.reshape(N_CORES, 2, 128).transpose(0, 2, 1)
    )
    return feat8_cores, feat16_cores, oh_cores, rec_cores, r_st


def kernel(features, batch, W1, b1, gamma, beta, W2, b2):
    feat8_cores, feat16_cores, oh_cores, rec_cores, r_st = _prep_inputs(features, batch)

    # fixed stage-1 stationaries: S[p, k, q] = 1 iff q == 16k + p//8
    p = np.arange(128)
    smat = np.zeros((128, K_SUB, 128), np.float16)
    for k in range(K_SUB):
        smat[p, k, 16 * k + p // G] = 1.0
    smat = smat.reshape(128, K_SUB * 128)

    ident = np.eye(128, dtype=np.float32)
    w1aug = np.concatenate(
        [np.asarray(W1, np.float32), np.asarray(b1, np.float32)[None, :]], axis=0
    )
    pvec = np.concatenate(
        [
            np.asarray(gamma, np.float32).ravel(),
            np.asarray(beta, np.float32).ravel(),
            np.asarray(W2, np.float32).ravel(),
            np.asarray(b2, np.float32).ravel(),
        ]
    )[None, :]

    nc = _build_program(r_st)
    in_maps = [
        {
            "feat8": feat8_cores[i],
            "feat16": feat16_cores[i],
            "smat": smat,
            "smat8": smat.astype(ml_dtypes.float8_e3m4),
            "oh": oh_cores[i],
            "ident": ident,
            "w1aug": w1aug,
            "pvec": pvec,
            "rec": rec_cores[i],
        }
        for i in range(N_CORES)
    ]
    res = run_bass_kernel_spmd(
        nc, in_maps, list(range(N_CORES)), trace=PROFILE, tmpdir=PROFILE_DIR
    )
    global LAST_RESULT
    LAST_RESULT = res
    out = np.concatenate(
        [res.results[i]["out"].reshape(SEGS_PER_CORE) for i in range(N_CORES)]
    )
    return out.reshape(NUM_GRAPHS, 1).astype(np.float32)
